# revision 1
# baseline (speedup 1.0000x reference)
"""AutoregressiveMlpMixer forward on 8 Trainium2 NeuronCores (Bass/Tile).

Strategy
- Pure data parallelism: 64 batch items -> 8 per core, weights replicated.
- The reverse cumsum over tokens is folded into tok_w1 on the host
  (suffix-sum then matmul == matmul with prefix-cumsum'd weights).
- LN2 / final-LN affine params are folded into the following matmul weights
  on the host. tok_b2 is dropped exactly (it is constant along the LN2
  normalization axis, so LN2 cancels it).
- Inter-block state X is kept TRANSPOSED ([channel, token] tiles): the
  channel-MLP second matmul then accumulates its 24 k-tiles into 6
  persistent PSUM banks while E/F stream weights fused per m-tile, so the
  gelu intermediate never materializes. LN1 re-transposes X on the PE.
- All matmuls run in float32r (~13 mantissa bits, full PE rate).
- Channel-MLP weights are streamed from HBM once per group of G=2 items.
"""

import sys

sys.path.insert(0, "/opt/trn_rl_repo")

import numpy as np

import concourse.bass as bass
import concourse.tile as tile
from concourse import bacc, masks, mybir

f32 = mybir.dt.float32
f32r = mybir.dt.float32r
AF = mybir.ActivationFunctionType
ALU = mybir.AluOpType

# Model dims (hardcoded per problem spec)
B, CIN, H, W = 64, 2, 32, 32
N = 256          # tokens
C = 768          # hidden dim
TOK = 512        # tokens_mlp_dim
CH = 3072        # channels_mlp_dim
L = 8            # blocks
K = 2048         # classes
EPS = 1e-5

NCORES = 8
IPC = B // NCORES    # items per core = 8
NT = N // 128        # 2 token tiles per item
CT = C // 128        # 6 channel tiles
MT = CH // 128       # 24 channel-mlp tiles
TT = TOK // 128      # 4 token-mlp tiles
CC = (512, 256)      # channel free-dim chunks for 768
CCO = (0, 512)
G = 2                # items per channel-MLP weight pass


def _ln_finish(nc, pool, st, magic_t, mode="dve"):
    """bn_aggr + rsqrt. st: [128, s, 6] bn_stats. Returns (mu, rstd) APs."""
    i32 = mybir.dt.int32
    mv = pool.tile([128, 2], f32, tag="ln_mv", bufs=8, name="mv")
    nc.vector.bn_aggr(out=mv, in_=st)
    v = mv[:, 1:2]
    if mode == "act":
        nc.scalar.activation(out=v, in_=v, func=AF.Abs_reciprocal_sqrt,
                             bias=magic_t[1], scale=1.0)
        return mv[:, 0:1], v
    eng = nc.gpsimd if mode == "pool" else nc.vector
    eng.tensor_scalar_add(v, v, float(EPS))
    iv = pool.tile([128, 1], i32, tag="rs_i", bufs=8, name="iv")
    eng.tensor_scalar(iv, v.bitcast(i32), 1, None,
                      ALU.logical_shift_right)
    eng.tensor_tensor(iv, magic_t[0], iv, ALU.subtract)
    y = iv.bitcast(f32)
    t = pool.tile([128, 1], f32, tag="rs_t", bufs=8, name="t")
    for _ in range(3):
        eng.tensor_mul(t, y, y)
        eng.tensor_mul(t, t, v)
        eng.tensor_scalar(t, t, -0.5, 1.5, ALU.mult, ALU.add)
        eng.tensor_mul(y, y, t)
    return mv[:, 0:1], y


def _ln_stats(nc, pool, x, magic_t, mode="dve"):
    """mean/rstd of x[128, C] over the free dim. Returns (mu, rstd) col APs."""
    st = pool.tile([128, 3, 6], f32, tag="ln_st", bufs=8, name="st")
    xg = x.rearrange("p (s q) -> p s q", s=3)
    for s in range(3):
        nc.vector.bn_stats(out=st[:, s, :], in_=xg[:, s, :])
    return _ln_finish(nc, pool, st, magic_t, mode)


def build(items=IPC, blocks=L, has_g1=False, has_b1=False, kchunk=24,
          rsqrt="act", pipelined=True):
    """Build the SPMD program for one core processing `items` batch items."""
    nc = bacc.Bacc("TRN2", target_bir_lowering=False, debug=False)

    # ---- DRAM tensors (names = in_map keys) ----
    pt = nc.dram_tensor("pt", [9, items * N], f32r, kind="ExternalInput")
    wq = nc.dram_tensor("wq", [9, C], f32r, kind="ExternalInput")
    bl = max(blocks, 1)
    tokw1c = nc.dram_tensor("tokw1c", [bl, NT, 128, TOK], f32r, kind="ExternalInput")
    tokw2 = nc.dram_tensor("tokw2", [bl, TT, 128, N], f32r, kind="ExternalInput")
    tokb1 = nc.dram_tensor("tokb1", [bl, 128, TT], f32, kind="ExternalInput")
    w1g = nc.dram_tensor("w1g", [bl, MT, 128, CT, 128], f32r, kind="ExternalInput")
    vb1 = nc.dram_tensor("vb1", [bl, 128, MT], f32, kind="ExternalInput")
    chw2 = nc.dram_tensor("chw2", [bl, MT, 128, C], f32r, kind="ExternalInput")
    chb2c = nc.dram_tensor("chb2c", [bl, 128, CT], f32, kind="ExternalInput")
    headwg = nc.dram_tensor("headwg", [CT, 128, K], f32r, kind="ExternalInput")
    headb = nc.dram_tensor("headb", [1, K], f32r, kind="ExternalInput")
    ln1g = nc.dram_tensor("ln1g", [bl, C], f32, kind="ExternalInput")
    ln1b = nc.dram_tensor("ln1b", [bl, C], f32, kind="ExternalInput")
    out = nc.dram_tensor("out", [items, K], f32, kind="ExternalOutput")

    n_groups = (items + G - 1) // G

    with tile.TileContext(nc) as tc:
        with tc.tile_pool(name="const", bufs=1) as const, \
             tc.tile_pool(name="xstate", bufs=1) as xstate:
            magic_i = const.tile([128, 1], mybir.dt.int32, name="magic_i")
            nc.vector.memset(magic_i, 0x5F3759DF)
            eps_col = const.tile([128, 1], f32, name="eps_col")
            nc.vector.memset(eps_col, EPS)
            magic_t = (magic_i, eps_col)
            ident = const.tile([128, 128], f32, name="ident")
            masks.make_identity(nc, ident)
            identr = const.tile([128, 128], f32r, name="identr")
            nc.vector.tensor_copy(identr, ident)

            # persistent state, TRANSPOSED: X[item][ct] = [128(c), N(tokens)]
            # f32r: PE transposes run at 1.5 cyc/row instead of 2.0
            X = [[xstate.tile([128, N], f32r, tag=f"x_{i}_{ct}",
                              name=f"x_{i}_{ct}")
                  for ct in range(CT)] for i in range(items)]

            # ---------------- stem (writes X transposed) ----------------
            with tc.tile_pool(name="stem", bufs=1) as stem, \
                 tc.tile_pool(name="ps_stem", bufs=4, space="PSUM") as ps_stem:
                ptt = stem.tile([9, items * N], f32r)
                nc.sync.dma_start(out=ptt, in_=pt[:, :])
                wqt = stem.tile([9, C], f32r)
                nc.sync.dma_start(out=wqt, in_=wq[:, :])
                nw_all = items * N
                nchunks = [(o, min(512, nw_all - o)) for o in range(0, nw_all, 512)]
                for ct in range(CT):
                    for (no, nn) in nchunks:
                        pss = ps_stem.tile([128, 512], f32, tag="pss", name="pss")
                        nc.tensor.matmul(pss[:, :nn],
                                         wqt[:, ct * 128:(ct + 1) * 128],
                                         ptt[:, no:no + nn],
                                         start=True, stop=True)
                        for j in range(0, nn, N):
                            i = (no + j) // N
                            nc.scalar.activation(out=X[i][ct],
                                                 in_=pss[:, j:j + N],
                                                 func=AF.Copy)

            # ---------------- mixer blocks ----------------
            with tc.tile_pool(name="tokw", bufs=2) as tokwp, \
                 tc.tile_pool(name="lnp", bufs=4) as lnp, \
                 tc.tile_pool(name="acts", bufs=1) as acts, \
                 tc.tile_pool(name="wstream", bufs=3) as wstream, \
                 tc.tile_pool(name="ps_mm", bufs=8, space="PSUM") as ps_mm:

                blk_w = {}

                def emit_tok_weights(l):
                    w = {}
                    w1c_t = tokwp.tile([128, NT, TOK], f32r, tag="w1c",
                                       name="w1c")
                    nc.sync.dma_start(out=w1c_t,
                                      in_=tokw1c[l].rearrange("k p t -> p k t"))
                    w2_t = tokwp.tile([128, TT, N], f32r, tag="w2", name="w2")
                    nc.sync.dma_start(out=w2_t,
                                      in_=tokw2[l].rearrange("k p n -> p k n"))
                    b1_t = tokwp.tile([128, TT], f32, tag="b1", name="b1")
                    nc.sync.dma_start(out=b1_t, in_=tokb1[l])
                    vb1_t = tokwp.tile([128, MT], f32, tag="vb1", name="vb1")
                    nc.sync.dma_start(out=vb1_t, in_=vb1[l])
                    chb2_t = tokwp.tile([128, CT], f32, tag="chb2", name="chb2")
                    nc.sync.dma_start(out=chb2_t, in_=chb2c[l])
                    w.update(w1c=w1c_t, w2=w2_t, b1=b1_t, vb1=vb1_t,
                             chb2=chb2_t)
                    if has_g1:
                        g1_t = tokwp.tile([128, C], f32, tag="g1", name="g1")
                        nc.sync.dma_start(
                            out=g1_t,
                            in_=ln1g.ap()[l:l + 1, :].partition_broadcast(128))
                        w["g1"] = g1_t
                    if has_b1:
                        b1v_t = tokwp.tile([128, C], f32, tag="b1v", name="b1v")
                        nc.sync.dma_start(
                            out=b1v_t,
                            in_=ln1b.ap()[l:l + 1, :].partition_broadcast(128))
                        w["b1v"] = b1v_t
                    return w

                def emit_AD(l, g):
                    """token-mix + LN stages for group g of block l -> Zt."""
                    if l not in blk_w:
                        blk_w[l] = emit_tok_weights(l)
                    w1c_t, w2_t, b1_t = (blk_w[l][k] for k in ("w1c", "w2", "b1"))
                    g1_t = blk_w[l].get("g1")
                    b1v_t = blk_w[l].get("b1v")
                    gitems = list(range(g * G, min((g + 1) * G, items)))
                    Zt = [acts.tile([128, G * N], f32r, tag=f"zt_{kc}",
                                    bufs=2, name=f"zt_{kc}")
                          for kc in range(CT)]
                    # ---- A for ALL group items first: transposes, stats,
                    # and rsqrt (adjacent rsqrts share one ACT table visit;
                    # the interleaved copies are table-set fillers) ----
                    pre = []
                    for i2, i in enumerate(gitems):
                        xn = [lnp.tile([128, C], f32, tag="xn", bufs=4,
                                       name="xn") for _ in range(NT)]
                        mus = []
                        for t in range(NT):
                            st = lnp.tile([128, 3, 6], f32, tag="ln_st",
                                          bufs=8, name="st")
                            for cg, cn in ((0, 4), (4, 2)):
                                ptr = ps_mm.tile([128, cn * 128], f32r,
                                                 tag="mm", name="ptrA")
                                for cc in range(cn):
                                    nc.tensor.transpose(
                                        ptr[:, cc * 128:(cc + 1) * 128],
                                        X[i][cg + cc][:, t * 128:(t + 1) * 128],
                                        identr)
                                nc.scalar.activation(
                                    out=xn[t][:, cg * 128:(cg + cn) * 128],
                                    in_=ptr, func=AF.Copy)
                                pgg = ptr.rearrange("p (s q) -> p s q", q=256)
                                for s in range(cn // 2):
                                    nc.vector.bn_stats(
                                        out=st[:, cg // 2 + s, :],
                                        in_=pgg[:, s, :])
                            mus.append(_ln_finish(nc, lnp, st, magic_t, rsqrt))
                        pre.append((xn, mus))
                    post = []
                    for i2, i in enumerate(gitems):
                        xn, mus = pre[i2]
                        Y = []
                        for t in range(NT):
                            mu, rstd = mus[t]
                            yt = lnp.tile([128, C], f32r, tag="y", bufs=4,
                                          name="yt")
                            for cw, co in zip(CC, CCO):
                                nc.vector.tensor_scalar(
                                    out=yt[:, co:co + cw],
                                    in0=xn[t][:, co:co + cw],
                                    scalar1=mu, scalar2=rstd,
                                    op0=ALU.subtract, op1=ALU.mult)
                            if has_g1:
                                nc.vector.tensor_mul(yt, yt, g1_t)
                            if has_b1:
                                nc.vector.tensor_add(yt, yt, b1v_t)
                            Y.append(yt)
                        # ---- B: y1 = gelu(w1cum^T @ Y + b1) ----
                        y1 = []
                        for mt in range(TT):
                            yg = lnp.tile([128, C], f32r, tag="y1g", bufs=8,
                                          name="yg")
                            for ci, (cw, co) in enumerate(zip(CC, CCO)):
                                pb = ps_mm.tile([128, 512], f32, tag="mm",
                                                name="pb")
                                for k in range(NT):
                                    nc.tensor.matmul(
                                        pb[:, :cw],
                                        w1c_t[:, k, mt * 128:(mt + 1) * 128],
                                        Y[k][:, co:co + cw],
                                        start=(k == 0), stop=(k == NT - 1))
                                nc.scalar.activation(
                                    out=yg[:, co:co + cw], in_=pb[:, :cw],
                                    func=AF.Gelu, bias=b1_t[:, mt:mt + 1],
                                    scale=1.0)
                            y1.append(yg)
                        # ---- C: y2 = w2^T @ y1, stats from PSUM ----
                        cstats = []
                        for t in range(NT):
                            y2t = lnp.tile([128, C], f32, tag="y2", bufs=4,
                                           name="y2t")
                            st = lnp.tile([128, 3, 6], f32, tag="ln_st",
                                          bufs=8, name="st")
                            for ci, (cw, co) in enumerate(zip(CC, CCO)):
                                pc = ps_mm.tile([128, 512], f32, tag="mm",
                                                name="pc")
                                for k in range(TT):
                                    nc.tensor.matmul(
                                        pc[:, :cw],
                                        w2_t[:, k, t * 128:(t + 1) * 128],
                                        y1[k][:, co:co + cw],
                                        start=(k == 0), stop=(k == TT - 1))
                                nc.scalar.activation(out=y2t[:, co:co + cw],
                                                     in_=pc[:, :cw],
                                                     func=AF.Copy)
                                # LN2 stats straight from PSUM
                                pg = pc[:, :cw].rearrange(
                                    "p (s q) -> p s q", q=256)
                                for s in range(cw // 256):
                                    nc.vector.bn_stats(
                                        out=st[:, 2 * ci + s, :],
                                        in_=pg[:, s, :])
                            cstats.append(
                                (y2t, _ln_finish(nc, lnp, st, magic_t,
                                                 rsqrt)))
                        post.append((i2, cstats))
                    # ---- LN2 apply + transpose into Zt, both items ----
                    for i2, cstats in post:
                        for t in range(NT):
                            y2t, (mu, rstd) = cstats[t]
                            zn = lnp.tile([128, C], f32r, tag="z", bufs=4,
                                          name="zn")
                            for cw, co in zip(CC, CCO):
                                nc.vector.tensor_scalar(
                                    out=zn[:, co:co + cw],
                                    in0=y2t[:, co:co + cw],
                                    scalar1=mu, scalar2=rstd,
                                    op0=ALU.subtract, op1=ALU.mult)
                            for cg, cn in ((0, 4), (4, 2)):
                                ptr = ps_mm.tile([128, cn * 128], f32r,
                                                 tag="mm", name="ptrT")
                                for cc in range(cn):
                                    nc.tensor.transpose(
                                        ptr[:, cc * 128:(cc + 1) * 128],
                                        zn[:, (cg + cc) * 128:
                                           (cg + cc + 1) * 128],
                                        identr)
                                for cc in range(cn):
                                    nc.vector.tensor_copy(
                                        Zt[cg + cc][:, i2 * N + t * 128:
                                                    i2 * N + (t + 1) * 128],
                                        ptr[:, cc * 128:(cc + 1) * 128])
                    return Zt

                def emit_EF(l, g, Zt, kchunk=kchunk):
                    """fused channel-MLP over m-tiles for group g of block l.

                    F accumulates in PSUM per k-chunk, then folds into the
                    SBUF state X (copy w/ bias on chunk 0, add afterwards) so
                    PSUM banks are only held transiently.
                    """
                    vb1_t = blk_w[l]["vb1"]
                    chb2_t = blk_w[l]["chb2"]
                    gitems = list(range(g * G, min((g + 1) * G, items)))
                    nw = len(gitems) * N
                    def emit_E(mt):
                        w1g_t = wstream.tile([128, CT, 128], f32r,
                                             tag="w1g", name="w1g_t")
                        nc.sync.dma_start(out=w1g_t, in_=w1g[l, mt])
                        pe = ps_mm.tile([128, 512], f32, tag="mm", name="pe")
                        for kc in range(CT):
                            nc.tensor.matmul(pe[:, :nw], w1g_t[:, kc, :],
                                             Zt[kc][:, :nw],
                                             start=(kc == 0),
                                             stop=(kc == CT - 1))
                        hg = acts.tile([128, G * N], f32r, tag="hg",
                                       bufs=3, name="hg")
                        nc.scalar.activation(out=hg[:, :nw], in_=pe[:, :nw],
                                             func=AF.Gelu,
                                             bias=vb1_t[:, mt:mt + 1],
                                             scale=1.0)
                        return hg

                    for k0 in range(0, MT, kchunk):
                        psF = [ps_mm.tile([128, G * N], f32, tag="mm",
                                          name=f"pf_{ct}") for ct in range(CT)]
                        for mt in range(k0, k0 + kchunk):
                            hg_cur = emit_E(mt)
                            w2c_t = wstream.tile([128, C], f32r, tag="w2c",
                                                 name="w2c_t")
                            nc.sync.dma_start(out=w2c_t, in_=chw2[l, mt])
                            for ct in range(CT):
                                nc.tensor.matmul(
                                    psF[ct][:, :nw],
                                    w2c_t[:, ct * 128:(ct + 1) * 128],
                                    hg_cur[:, :nw],
                                    start=(mt == k0),
                                    stop=(mt == k0 + kchunk - 1))
                        for ct in range(CT):
                            for i2, i in enumerate(gitems):
                                src = psF[ct][:, i2 * N:(i2 + 1) * N]
                                if k0 == 0:
                                    nc.scalar.activation(
                                        out=X[i][ct], in_=src,
                                        func=AF.Identity,
                                        bias=chb2_t[:, ct:ct + 1], scale=1.0)
                                else:
                                    nc.vector.tensor_add(X[i][ct], X[i][ct],
                                                         src)

                # software-pipelined emission: A-D of step s+1 lands before
                # E/F of step s so the scheduler can fill LN-latency bubbles.
                seq = [(l, g) for l in range(blocks) for g in range(n_groups)]
                zts = {}
                if pipelined:
                    if seq:
                        zts[seq[0]] = emit_AD(*seq[0])
                    for idx, key in enumerate(seq):
                        if idx + 1 < len(seq):
                            nkey = seq[idx + 1]
                            zts[nkey] = emit_AD(*nkey)
                        emit_EF(*key, zts.pop(key))
                else:
                    for key in seq:
                        emit_EF(*key, emit_AD(*key))
            # ---------------- final LN + token-mean + head ----------------
            with tc.tile_pool(name="headp", bufs=1) as headp, \
                 tc.tile_pool(name="lnf", bufs=4) as lnf, \
                 tc.tile_pool(name="ps_h", bufs=2, space="PSUM") as ps_h:
                invn_f = headp.tile([128, 2], f32)
                nc.vector.memset(invn_f, 1.0 / N)
                invn_col = headp.tile([128, 2], f32r)
                nc.vector.tensor_copy(invn_col, invn_f)
                ones8_f = headp.tile([1, items], f32)
                nc.vector.memset(ones8_f, 1.0)
                ones8 = headp.tile([1, items], f32r)
                nc.vector.tensor_copy(ones8, ones8_f)
                xmall = headp.tile([128, CT, items], f32r)
                for i in range(items):
                    xf = [lnf.tile([128, C], f32, tag="xf", bufs=4, name="xf")
                          for _ in range(NT)]
                    for ct in range(CT):
                        for t in range(NT):
                            ptr = ps_h.tile([128, 128], f32r, tag="pth",
                                            name="ptrH")
                            nc.tensor.transpose(
                                ptr, X[i][ct][:, t * 128:(t + 1) * 128], identr)
                            nc.vector.tensor_copy(
                                xf[t][:, ct * 128:(ct + 1) * 128], ptr)
                    xh = []
                    for t in range(NT):
                        mu, rstd = _ln_stats(nc, lnf, xf[t], magic_t, rsqrt)
                        xht = lnf.tile([128, C], f32r, tag="xh", bufs=4,
                                       name="xht")
                        nc.vector.tensor_scalar(
                            out=xht, in0=xf[t], scalar1=mu, scalar2=rstd,
                            op0=ALU.subtract, op1=ALU.mult)
                        xh.append(xht)
                    for ct in range(CT):
                        pxm = ps_h.tile([128, 2], f32, tag="pxm", name="pxm")
                        for t in range(NT):
                            nc.tensor.matmul(pxm,
                                             xh[t][:, ct * 128:(ct + 1) * 128],
                                             invn_col,
                                             start=(t == 0), stop=(t == NT - 1))
                        nc.scalar.activation(out=xmall[:, ct, i:i + 1],
                                             in_=pxm[:, 0:1], func=AF.Copy)
                hb_t = headp.tile([1, K], f32r)
                nc.sync.dma_start(out=hb_t, in_=headb[:, :])
                outsb = headp.tile([items, K], f32)
                for jc in range(K // 512):
                    ph = ps_h.tile([items, 512], f32, tag="ph", name="ph")
                    for ct in range(CT):
                        hw_t = headp.tile([128, 512], f32r, tag="hw", bufs=4,
                                          name="hw_t")
                        nc.sync.dma_start(
                            out=hw_t, in_=headwg[ct, :, jc * 512:(jc + 1) * 512])
                        nc.tensor.matmul(ph, xmall[:, ct, :items], hw_t,
                                         start=(ct == 0), stop=False)
                    nc.tensor.matmul(ph, ones8, hb_t[:, jc * 512:(jc + 1) * 512],
                                     start=False, stop=True)
                    nc.scalar.activation(out=outsb[:, jc * 512:(jc + 1) * 512],
                                         in_=ph, func=AF.Copy)
                nc.sync.dma_start(out=out[:, :], in_=outsb)

    nc.compile()
    return nc


# ---------------------------------------------------------------------------
# host-side preprocessing
# ---------------------------------------------------------------------------

def prep_inputs(inputs, stem_w, stem_b, ln1_g, ln1_b, tok_w1, tok_b1, tok_w2,
                tok_b2, ln2_g, ln2_b, ch_w1, ch_b1, ch_w2, ch_b2, lnf_g, lnf_b,
                head_w, head_b, items=IPC, blocks=L):
    """Returns (shared_map, per_core_list, flags)."""
    f = np.float32
    inputs = np.asarray(inputs, f)
    # patches: (B, CIN, 16, 2, 16, 2) -> (B, n=256, q=8); +ones row -> (B,9,256)
    x = inputs.reshape(B, CIN, H // 2, 2, W // 2, 2).transpose(0, 2, 4, 1, 3, 5)
    x = x.reshape(B, N, CIN * 4)
    ptA = np.concatenate([x.transpose(0, 2, 1),
                          np.ones((B, 1, N), f)], axis=1)  # (B, 9, 256)

    wq = np.concatenate([np.asarray(stem_w, f).reshape(C, 8).T,
                         np.asarray(stem_b, f)[None, :]], axis=0)  # (9, C)

    blocks = max(blocks, 1)
    w1cum = np.cumsum(np.asarray(tok_w1, f), axis=1)[:blocks]        # (L, N, TOK)
    tokw1c = np.ascontiguousarray(w1cum.reshape(blocks, NT, 128, TOK))
    tokw2 = np.ascontiguousarray(np.asarray(tok_w2, f)[:blocks]
                                 .reshape(blocks, TT, 128, N))
    tokb1 = np.ascontiguousarray(np.asarray(tok_b1, f)[:blocks]
                                 .reshape(blocks, TT, 128).transpose(0, 2, 1))

    g2 = np.asarray(ln2_g, f)[:blocks]
    b2 = np.asarray(ln2_b, f)[:blocks]
    cw1 = np.asarray(ch_w1, f)[:blocks]
    w1g_full = g2[:, :, None] * cw1                                   # (L, C, CH)
    w1g = np.ascontiguousarray(
        w1g_full.reshape(blocks, CT, 128, MT, 128).transpose(0, 3, 2, 1, 4))
    v = np.einsum("lc,lcm->lm", b2, cw1) + np.asarray(ch_b1, f)[:blocks]
    vb1 = np.ascontiguousarray(v.reshape(blocks, MT, 128).transpose(0, 2, 1))
    chw2 = np.ascontiguousarray(np.asarray(ch_w2, f)[:blocks]
                                .reshape(blocks, MT, 128, C))
    chb2c = np.ascontiguousarray(np.asarray(ch_b2, f)[:blocks]
                                 .reshape(blocks, CT, 128).transpose(0, 2, 1))

    gf = np.asarray(lnf_g, f)
    bf = np.asarray(lnf_b, f)
    hw = np.asarray(head_w, f)
    headwg = np.ascontiguousarray((gf[:, None] * hw).reshape(CT, 128, K))
    headb = (bf @ hw + np.asarray(head_b, f)).reshape(1, K).astype(f)

    ln1g = np.ascontiguousarray(np.asarray(ln1_g, f)[:blocks])
    ln1b = np.ascontiguousarray(np.asarray(ln1_b, f)[:blocks])
    has_g1 = not np.all(ln1g == 1.0)
    has_b1 = not np.all(ln1b == 0.0)

    shared = dict(wq=wq, tokw1c=tokw1c, tokw2=tokw2, tokb1=tokb1, w1g=w1g,
                  vb1=vb1, chw2=chw2, chb2c=chb2c, headwg=headwg, headb=headb,
                  ln1g=ln1g, ln1b=ln1b)
    shared = {k: np.ascontiguousarray(v, f) for k, v in shared.items()}

    per_core = []
    for c in range(NCORES):
        sel = ptA[c * IPC:(c + 1) * IPC][:items]  # (items, 9, 256)
        ptc = np.ascontiguousarray(sel.transpose(1, 0, 2).reshape(9, items * N))
        per_core.append(dict(pt=ptc))
    return shared, per_core, dict(has_g1=has_g1, has_b1=has_b1)


_CACHE = {}


def kernel(**inputs):
    from concourse.bass_utils import run_bass_kernel_spmd
    shared, per_core, flags = prep_inputs(**inputs)
    key = (flags["has_g1"], flags["has_b1"])
    if key not in _CACHE:
        _CACHE[key] = build(has_g1=flags["has_g1"], has_b1=flags["has_b1"])
    nc = _CACHE[key]
    in_maps = [{**shared, **pc} for pc in per_core]
    res = run_bass_kernel_spmd(nc, in_maps, core_ids=list(range(NCORES)))
    outs = [r["out"] for r in res.results]
    return np.concatenate(outs, axis=0).astype(np.float32)



# revision 29
# speedup vs baseline: 1.2203x; 1.2203x over previous
"""AutoregressiveMlpMixer forward on 8 Trainium2 NeuronCores (Bass/Tile).

Strategy
- Pure data parallelism: 64 batch items -> 8 per core, weights replicated.
- The reverse cumsum over tokens is folded into tok_w1 on the host
  (suffix-sum then matmul == matmul with prefix-cumsum'd weights).
- LN2 / final-LN affine params are folded into the following matmul weights
  on the host. tok_b2 is dropped exactly (it is constant along the LN2
  normalization axis, so LN2 cancels it).
- Inter-block state X is kept TRANSPOSED ([channel, token] tiles): the
  channel-MLP second matmul then accumulates its 24 k-tiles into 6
  persistent PSUM banks while E/F stream weights fused per m-tile, so the
  gelu intermediate never materializes. LN1 re-transposes X on the PE.
- All matmuls run in float32r (~13 mantissa bits, full PE rate).
- Channel-MLP weights are streamed from HBM once per group of G=2 items.
"""

import sys

sys.path.insert(0, "/opt/trn_rl_repo")

import numpy as np

import concourse.bass as bass
import concourse.tile as tile
from concourse import bacc, masks, mybir

f32 = mybir.dt.float32
f32r = mybir.dt.float32r
AF = mybir.ActivationFunctionType
ALU = mybir.AluOpType

# Model dims (hardcoded per problem spec)
B, CIN, H, W = 64, 2, 32, 32
N = 256          # tokens
C = 768          # hidden dim
TOK = 512        # tokens_mlp_dim
CH = 3072        # channels_mlp_dim
L = 8            # blocks
K = 2048         # classes
EPS = 1e-5

NCORES = 8
IPC = B // NCORES    # items per core = 8
NT = N // 128        # 2 token tiles per item
CT = C // 128        # 6 channel tiles
MT = CH // 128       # 24 channel-mlp tiles
TT = TOK // 128      # 4 token-mlp tiles
CC = (512, 256)      # channel free-dim chunks for 768
CCO = (0, 512)
G = 2                # items per channel-MLP weight pass


def _ln_finish(nc, pool, st, magic_t, mode="dve"):
    """bn_aggr + rsqrt. st: [128, s, 6] bn_stats. Returns (mu, rstd) APs."""
    return _ln_finish_batch(nc, pool, [st], magic_t, mode)[0]


def _ln_finish_batch(nc, pool, sts, magic_t, mode="dve"):
    """Batched bn_aggr + Newton rsqrt for n<=4 LN sites on the DVE.

    sts: list of [128, 3, 6] bn_stats tiles. Returns [(mu, rstd)] col APs.
    Batching amortizes the per-op overhead of the 9-op Newton chain."""
    i32 = mybir.dt.int32
    n = len(sts)
    # [128, 2, n]: row 0 = means, row 1 = vars -> var row is CONTIGUOUS so
    # the bitcast in the Newton iteration below is legal.
    mv = pool.tile([128, 2, n], f32, tag=f"ln_mv{n}", bufs=4, name="mv")
    for s, st in enumerate(sts):
        nc.vector.bn_aggr(out=mv[:, :, s], in_=st)
    v = mv[:, 1, :]
    if mode == "act":
        nc.scalar.activation(out=v, in_=v, func=AF.Abs_reciprocal_sqrt,
                             bias=magic_t[1], scale=1.0)
        return [(mv[:, 0, s:s + 1], v[:, s:s + 1]) for s in range(n)]
    eng = nc.gpsimd if mode == "pool" else nc.vector
    eng.tensor_scalar_add(v, v, float(EPS))
    iv = pool.tile([128, n], i32, tag=f"rs_i{n}", bufs=4, name="iv")
    eng.tensor_scalar(iv, v.bitcast(i32), 1, None,
                      ALU.logical_shift_right)
    eng.tensor_tensor(iv, magic_t[0][:, :n], iv, ALU.subtract)
    y = iv.bitcast(f32)
    t = pool.tile([128, n], f32, tag=f"rs_t{n}", bufs=4, name="t")
    for _ in range(3):
        eng.tensor_mul(t, y, y)
        eng.tensor_mul(t, t, v)
        eng.tensor_scalar(t, t, -0.5, 1.5, ALU.mult, ALU.add)
        eng.tensor_mul(y, y, t)
    return [(mv[:, 0, s:s + 1], y[:, s:s + 1]) for s in range(n)]


def _ln_stats(nc, pool, x, magic_t, mode="dve"):
    """mean/rstd of x[128, C] over the free dim. Returns (mu, rstd) col APs."""
    st = pool.tile([128, 3, 6], f32, tag="ln_st", bufs=8, name="st")
    xg = x.rearrange("p (s q) -> p s q", s=3)
    for s in range(3):
        nc.vector.bn_stats(out=st[:, s, :], in_=xg[:, s, :])
    return _ln_finish(nc, pool, st, magic_t, mode)


def build(items=IPC, blocks=L, has_g1=False, has_b1=False, kchunk=24,
          rsqrt="dve", pipelined=True):
    """Build the SPMD program for one core processing `items` batch items."""
    nc = bacc.Bacc("TRN2", target_bir_lowering=False, debug=False)

    # ---- DRAM tensors (names = in_map keys) ----
    pt = nc.dram_tensor("pt", [9, items * N], f32r, kind="ExternalInput")
    wq = nc.dram_tensor("wq", [9, C], f32r, kind="ExternalInput")
    bl = max(blocks, 1)
    tokw1c = nc.dram_tensor("tokw1c", [bl, NT, 128, TOK], f32r, kind="ExternalInput")
    tokw2 = nc.dram_tensor("tokw2", [bl, TT, 128, N], f32r, kind="ExternalInput")
    tokb1 = nc.dram_tensor("tokb1", [bl, 128, TT], f32, kind="ExternalInput")
    w1g = nc.dram_tensor("w1g", [bl, MT // 2, 128, 2, CT, 128], f32r,
                         kind="ExternalInput")
    vb1 = nc.dram_tensor("vb1", [bl, 128, MT], f32, kind="ExternalInput")
    chw2 = nc.dram_tensor("chw2", [bl, MT // 2, 128, 2, C], f32r,
                          kind="ExternalInput")
    chb2c = nc.dram_tensor("chb2c", [bl, 128, CT], f32, kind="ExternalInput")
    headwg = nc.dram_tensor("headwg", [CT, 128, K], f32r, kind="ExternalInput")
    headb = nc.dram_tensor("headb", [1, K], f32r, kind="ExternalInput")
    ln1g = nc.dram_tensor("ln1g", [bl, C], f32, kind="ExternalInput")
    ln1b = nc.dram_tensor("ln1b", [bl, C], f32, kind="ExternalInput")
    out = nc.dram_tensor("out", [items, K], f32, kind="ExternalOutput")

    n_groups = (items + G - 1) // G

    with tile.TileContext(nc) as tc:
        with tc.tile_pool(name="const", bufs=1) as const, \
             tc.tile_pool(name="xstate", bufs=1) as xstate:
            magic_i = const.tile([128, 4], mybir.dt.int32, name="magic_i")
            nc.vector.memset(magic_i, 0x5F3759DF)
            eps_col = const.tile([128, 1], f32, name="eps_col")
            nc.vector.memset(eps_col, EPS)
            magic_t = (magic_i, eps_col)
            ident = const.tile([128, 128], f32, name="ident")
            masks.make_identity(nc, ident)
            identr = const.tile([128, 128], f32r, name="identr")
            nc.vector.tensor_copy(identr, ident)
            bf16 = mybir.dt.bfloat16
            identb = const.tile([128, 128], bf16, name="identb")
            nc.vector.tensor_copy(identb, ident)

            # persistent state, TRANSPOSED: X[item][ct] = [128(c), N(tokens)]
            # bf16: PE transposes run at 1.0 cyc/row and SBUF halves (the
            # ~0.2% storage noise costs ~6e-3 final rel err, within budget)
            X = [[xstate.tile([128, N], bf16, tag=f"x_{i}_{ct}",
                              name=f"x_{i}_{ct}")
                  for ct in range(CT)] for i in range(items)]

            # ---------------- stem (writes X transposed) ----------------
            with tc.tile_pool(name="stem", bufs=1) as stem, \
                 tc.tile_pool(name="ps_stem", bufs=4, space="PSUM") as ps_stem:
                ptt = stem.tile([9, items * N], f32r)
                nc.sync.dma_start(out=ptt, in_=pt[:, :])
                wqt = stem.tile([9, C], f32r)
                nc.sync.dma_start(out=wqt, in_=wq[:, :])
                nw_all = items * N
                nchunks = [(o, min(512, nw_all - o)) for o in range(0, nw_all, 512)]
                for ct in range(CT):
                    for (no, nn) in nchunks:
                        pss = ps_stem.tile([128, 512], f32, tag="pss", name="pss")
                        nc.tensor.matmul(pss[:, :nn],
                                         wqt[:, ct * 128:(ct + 1) * 128],
                                         ptt[:, no:no + nn],
                                         start=True, stop=True)
                        for j in range(0, nn, N):
                            i = (no + j) // N
                            nc.scalar.activation(out=X[i][ct],
                                                 in_=pss[:, j:j + N],
                                                 func=AF.Copy)

            # ---------------- mixer blocks ----------------
            with tc.tile_pool(name="tokw", bufs=2) as tokwp, \
                 tc.tile_pool(name="lnp", bufs=4) as lnp, \
                 tc.tile_pool(name="acts", bufs=1) as acts, \
                 tc.tile_pool(name="wstream", bufs=4) as wstream, \
                 tc.tile_pool(name="ps_mm", bufs=8, space="PSUM") as ps_mm:

                blk_w = {}

                def emit_tok_weights(l):
                    w = {}
                    w1c_t = tokwp.tile([128, NT, TOK], f32r, tag="w1c",
                                       name="w1c")
                    nc.sync.dma_start(out=w1c_t,
                                      in_=tokw1c[l].rearrange("k p t -> p k t"))
                    w2_t = tokwp.tile([128, TT, N], f32r, tag="w2", name="w2")
                    nc.sync.dma_start(out=w2_t,
                                      in_=tokw2[l].rearrange("k p n -> p k n"))
                    b1_t = tokwp.tile([128, TT], f32, tag="b1", name="b1")
                    nc.sync.dma_start(out=b1_t, in_=tokb1[l])
                    vb1_t = tokwp.tile([128, MT], f32, tag="vb1", name="vb1")
                    nc.sync.dma_start(out=vb1_t, in_=vb1[l])
                    chb2_t = tokwp.tile([128, CT], f32, tag="chb2", name="chb2")
                    nc.sync.dma_start(out=chb2_t, in_=chb2c[l])
                    w.update(w1c=w1c_t, w2=w2_t, b1=b1_t, vb1=vb1_t,
                             chb2=chb2_t)
                    if has_g1:
                        g1_t = tokwp.tile([128, C], f32, tag="g1", name="g1")
                        nc.sync.dma_start(
                            out=g1_t,
                            in_=ln1g.ap()[l:l + 1, :].partition_broadcast(128))
                        w["g1"] = g1_t
                    if has_b1:
                        b1v_t = tokwp.tile([128, C], f32, tag="b1v", name="b1v")
                        nc.sync.dma_start(
                            out=b1v_t,
                            in_=ln1b.ap()[l:l + 1, :].partition_broadcast(128))
                        w["b1v"] = b1v_t
                    return w

                def emit_A(l, g):
                    """LN1 for group g: transposes + stats + batched Newton
                    + apply -> Y tiles. Emitted 2 steps ahead of its EF so
                    the DVE Newton chain never gates the PE stream."""
                    if l not in blk_w:
                        blk_w[l] = emit_tok_weights(l)
                    g1_t = blk_w[l].get("g1")
                    b1v_t = blk_w[l].get("b1v")
                    gitems = list(range(g * G, min((g + 1) * G, items)))
                    pre = []
                    sts = []
                    for i2, i in enumerate(gitems):
                        xn = [lnp.tile([128, C], bf16, tag="xn", bufs=4,
                                       name="xn") for _ in range(NT)]
                        for t in range(NT):
                            st = lnp.tile([128, 3, 6], f32, tag="ln_st",
                                          bufs=12, name="st")
                            for cg, cn in ((0, 4), (4, 2)):
                                ptr = ps_mm.tile([128, cn * 128], bf16,
                                                 tag="mm", name="ptrA")
                                for cc in range(cn):
                                    nc.tensor.transpose(
                                        ptr[:, cc * 128:(cc + 1) * 128],
                                        X[i][cg + cc][:, t * 128:(t + 1) * 128],
                                        identb)
                                nc.scalar.activation(
                                    out=xn[t][:, cg * 128:(cg + cn) * 128],
                                    in_=ptr, func=AF.Copy)
                                pgg = ptr.rearrange("p (s q) -> p s q", q=256)
                                for s in range(cn // 2):
                                    nc.vector.bn_stats(
                                        out=st[:, cg // 2 + s, :],
                                        in_=pgg[:, s, :])
                            sts.append(st)
                        pre.append(xn)
                    musall = _ln_finish_batch(nc, lnp, sts, magic_t, rsqrt)
                    Ys = []
                    for i2, i in enumerate(gitems):
                        xn = pre[i2]
                        Y = []
                        for t in range(NT):
                            mu, rstd = musall[i2 * NT + t]
                            yt = lnp.tile([128, C], f32r, tag="y", bufs=8,
                                          name="yt")
                            for cw, co in zip(CC, CCO):
                                nc.vector.tensor_scalar(
                                    out=yt[:, co:co + cw],
                                    in0=xn[t][:, co:co + cw],
                                    scalar1=mu, scalar2=rstd,
                                    op0=ALU.subtract, op1=ALU.mult)
                            if has_g1:
                                nc.vector.tensor_mul(yt, yt, g1_t)
                            if has_b1:
                                nc.vector.tensor_add(yt, yt, b1v_t)
                            Y.append(yt)
                        Ys.append(Y)
                    return Ys

                def emit_BC(l, g, Ys):
                    """token-mix + LN2 stats for group g -> (y2 tiles, rstds).
                    The LN2 apply/transpose (emit_DZ) is emitted an iteration
                    later so its Newton chain never gates the PE stream."""
                    w1c_t, w2_t, b1_t = (blk_w[l][k] for k in ("w1c", "w2", "b1"))
                    gitems = list(range(g * G, min((g + 1) * G, items)))
                    out = []
                    y1s = []
                    # ---- B for ALL items first: the last B-gelu's latency
                    # then hides under the other item's C matmuls ----
                    for i2, i in enumerate(gitems):
                        Y = Ys[i2]
                        y1 = []
                        for mt in range(TT):
                            yg = lnp.tile([128, C], f32r, tag="y1g", bufs=8,
                                          name="yg")
                            for ci, (cw, co) in enumerate(zip(CC, CCO)):
                                pb = ps_mm.tile([128, 512], f32, tag="mm",
                                                name="pb")
                                for k in range(NT):
                                    nc.tensor.matmul(
                                        pb[:, :cw],
                                        w1c_t[:, k, mt * 128:(mt + 1) * 128],
                                        Y[k][:, co:co + cw],
                                        start=(k == 0), stop=(k == NT - 1))
                                nc.scalar.activation(
                                    out=yg[:, co:co + cw], in_=pb[:, :cw],
                                    func=AF.Gelu, bias=b1_t[:, mt:mt + 1],
                                    scale=1.0)
                            y1.append(yg)
                        y1s.append(y1)
                    for i2, i in enumerate(gitems):
                        y1 = y1s[i2]
                        # ---- C: y2 = w2^T @ y1, stats from PSUM ----
                        cpost = []
                        csts = []
                        for t in range(NT):
                            y2t = lnp.tile([128, C], f32, tag="y2", bufs=6,
                                           name="y2t")
                            st = lnp.tile([128, 3, 6], f32, tag="ln_st",
                                          bufs=12, name="st")
                            for ci, (cw, co) in enumerate(zip(CC, CCO)):
                                pc = ps_mm.tile([128, 512], f32, tag="mm",
                                                name="pc")
                                for k in range(TT):
                                    nc.tensor.matmul(
                                        pc[:, :cw],
                                        w2_t[:, k, t * 128:(t + 1) * 128],
                                        y1[k][:, co:co + cw],
                                        start=(k == 0), stop=(k == TT - 1))
                                nc.scalar.activation(out=y2t[:, co:co + cw],
                                                     in_=pc[:, :cw],
                                                     func=AF.Copy)
                                # LN2 stats straight from PSUM
                                pg = pc[:, :cw].rearrange(
                                    "p (s q) -> p s q", q=256)
                                for s in range(cw // 256):
                                    nc.vector.bn_stats(
                                        out=st[:, 2 * ci + s, :],
                                        in_=pg[:, s, :])
                            cpost.append(y2t)
                            csts.append(st)
                        cmus = _ln_finish_batch(nc, lnp, csts, magic_t, rsqrt)
                        # LN2 apply now (its Newton latency hides under the
                        # concurrent EF); the transposes wait for emit_DZ.
                        zns = []
                        for t in range(NT):
                            mu, rstd = cmus[t]
                            zn = lnp.tile([128, C], bf16, tag="z", bufs=8,
                                          name="zn")
                            for cw, co in zip(CC, CCO):
                                nc.vector.tensor_scalar(
                                    out=zn[:, co:co + cw],
                                    in0=cpost[t][:, co:co + cw],
                                    scalar1=mu, scalar2=rstd,
                                    op0=ALU.subtract, op1=ALU.mult)
                            zns.append(zn)
                        out.append(zns)
                    return out

                def emit_DZ(l, g, bc):
                    """Transpose LN2 output into Zt. Emitted FIRST in its
                    iteration, one after emit_BC: the zn tiles are ready, so
                    the PE transposes and DVE copies fire immediately."""
                    Zt = acts.tile([128, CT, G * N], f32r, tag="zt",
                                   bufs=2, name="zt")
                    for i2, zns in enumerate(bc):
                        for t in range(NT):
                            zn = zns[t]
                            for cg, cn in ((0, 4), (4, 2)):
                                ptr = ps_mm.tile([128, cn * 128], bf16,
                                                 tag="mm", name="ptrT")
                                for cc in range(cn):
                                    nc.tensor.transpose(
                                        ptr[:, cc * 128:(cc + 1) * 128],
                                        zn[:, (cg + cc) * 128:
                                           (cg + cc + 1) * 128],
                                        identb)
                                nc.vector.tensor_copy(
                                    Zt[:, cg:cg + cn,
                                       i2 * N + t * 128:i2 * N + (t + 1) * 128],
                                    ptr.rearrange("p (c q) -> p c q", q=128))
                    return Zt

                def emit_EF(l, g, Zt, kchunk=kchunk):
                    """fused channel-MLP over m-tiles for group g of block l.

                    F accumulates in PSUM per k-chunk, then folds into the
                    SBUF state X (copy w/ bias on chunk 0, add afterwards) so
                    PSUM banks are only held transiently.
                    """
                    vb1_t = blk_w[l]["vb1"]
                    chb2_t = blk_w[l]["chb2"]
                    gitems = list(range(g * G, min((g + 1) * G, items)))
                    nw = len(gitems) * N

                    for k0 in range(0, MT, kchunk):
                        psF = [ps_mm.tile([128, G * N], f32, tag="mm",
                                          name=f"pf_{ct}") for ct in range(CT)]
                        for pp in range(k0 // 2, (k0 + kchunk) // 2):
                            w1g_t = wstream.tile([128, 2, CT, 128], f32r,
                                                 tag="w1g", name="w1g_t")
                            nc.sync.dma_start(out=w1g_t, in_=w1g[l, pp])
                            w2c_t = wstream.tile([128, 2, C], f32r,
                                                 tag="w2c", name="w2c_t")
                            nc.sync.dma_start(out=w2c_t, in_=chw2[l, pp])
                            # E,E then F,F per pair: each gelu's latency
                            # hides under the other matmul of the pair.
                            hgs = []
                            for j in (0, 1):
                                mt = 2 * pp + j
                                pe = ps_mm.tile([128, 512], f32, tag="mm",
                                                name="pe")
                                for kc in range(CT):
                                    nc.tensor.matmul(pe[:, :nw],
                                                     w1g_t[:, j, kc, :],
                                                     Zt[:, kc, :nw],
                                                     start=(kc == 0),
                                                     stop=(kc == CT - 1))
                                hg_cur = acts.tile([128, G * N], f32r,
                                                   tag="hg", bufs=3,
                                                   name="hg")
                                nc.scalar.activation(out=hg_cur[:, :nw],
                                                     in_=pe[:, :nw],
                                                     func=AF.Gelu,
                                                     bias=vb1_t[:, mt:mt + 1],
                                                     scale=1.0)
                                hgs.append(hg_cur)
                            for j in (0, 1):
                                mt = 2 * pp + j
                                for ct in range(CT):
                                    nc.tensor.matmul(
                                        psF[ct][:, :nw],
                                        w2c_t[:, j, ct * 128:(ct + 1) * 128],
                                        hgs[j][:, :nw],
                                        start=(mt == k0),
                                        stop=(mt == k0 + kchunk - 1))
                        for ct in range(CT):
                            for i2, i in enumerate(gitems):
                                src = psF[ct][:, i2 * N:(i2 + 1) * N]
                                if k0 == 0:
                                    nc.scalar.activation(
                                        out=X[i][ct], in_=src,
                                        func=AF.Identity,
                                        bias=chb2_t[:, ct:ct + 1], scale=1.0)
                                else:
                                    nc.vector.tensor_add(X[i][ct], X[i][ct],
                                                         src)

                # software-pipelined emission: per-engine instruction streams
                # are in-order, so A (whose LN Newton chain gates B) is
                # emitted TWO steps ahead and BCD one step ahead of EF.
                seq = [(l, g) for l in range(blocks) for g in range(n_groups)]
                ys, bcs = {}, {}
                # A(s+la) reads X written by EF(s+la-n_groups), so the A
                # lookahead must stay below n_groups (program order = dep
                # order for the tile framework). Per iteration the emission
                # order is A(s+2), BC(s+1), DZ(s), EF(s): every cross-engine
                # latency chain (LN Newton on DVE) resolves a full EF phase
                # before the PE stream needs its result.
                la = min(2, n_groups - 1) if pipelined else 0
                lb = min(1, la)
                if la > 0:
                    for k in range(min(la, len(seq))):
                        ys[seq[k]] = emit_A(*seq[k])
                    for k in range(min(lb, len(seq))):
                        bcs[seq[k]] = emit_BC(*seq[k], ys.pop(seq[k]))
                    for idx, key in enumerate(seq):
                        zt = emit_DZ(*key, bcs[key])
                        if idx + la < len(seq):
                            nkey = seq[idx + la]
                            ys[nkey] = emit_A(*nkey)
                        if idx + lb < len(seq):
                            nkey = seq[idx + lb]
                            if nkey not in bcs:
                                bcs[nkey] = emit_BC(*nkey, ys.pop(nkey))
                        bcs.pop(key)
                        emit_EF(*key, zt)
                else:
                    for key in seq:
                        emit_EF(*key, emit_DZ(*key, emit_BC(*key,
                                                            emit_A(*key))))
            # ---------------- final LN + token-mean + head ----------------
            with tc.tile_pool(name="headp", bufs=1) as headp, \
                 tc.tile_pool(name="lnf", bufs=4) as lnf, \
                 tc.tile_pool(name="ps_h", bufs=2, space="PSUM") as ps_h:
                invn_f = headp.tile([128, 2], f32)
                nc.vector.memset(invn_f, 1.0 / N)
                invn_col = headp.tile([128, 2], f32r)
                nc.vector.tensor_copy(invn_col, invn_f)
                ones8_f = headp.tile([1, items], f32)
                nc.vector.memset(ones8_f, 1.0)
                ones8 = headp.tile([1, items], f32r)
                nc.vector.tensor_copy(ones8, ones8_f)
                xmall = headp.tile([128, CT, items], f32r)
                for i in range(items):
                    xf = [lnf.tile([128, C], f32, tag="xf", bufs=4, name="xf")
                          for _ in range(NT)]
                    for ct in range(CT):
                        for t in range(NT):
                            ptr = ps_h.tile([128, 128], bf16, tag="pth",
                                            name="ptrH")
                            nc.tensor.transpose(
                                ptr, X[i][ct][:, t * 128:(t + 1) * 128], identb)
                            nc.vector.tensor_copy(
                                xf[t][:, ct * 128:(ct + 1) * 128], ptr)
                    xh = []
                    for t in range(NT):
                        mu, rstd = _ln_stats(nc, lnf, xf[t], magic_t, rsqrt)
                        xht = lnf.tile([128, C], f32r, tag="xh", bufs=4,
                                       name="xht")
                        nc.vector.tensor_scalar(
                            out=xht, in0=xf[t], scalar1=mu, scalar2=rstd,
                            op0=ALU.subtract, op1=ALU.mult)
                        xh.append(xht)
                    for ct in range(CT):
                        pxm = ps_h.tile([128, 2], f32, tag="pxm", name="pxm")
                        for t in range(NT):
                            nc.tensor.matmul(pxm,
                                             xh[t][:, ct * 128:(ct + 1) * 128],
                                             invn_col,
                                             start=(t == 0), stop=(t == NT - 1))
                        nc.scalar.activation(out=xmall[:, ct, i:i + 1],
                                             in_=pxm[:, 0:1], func=AF.Copy)
                hb_t = headp.tile([1, K], f32r)
                nc.sync.dma_start(out=hb_t, in_=headb[:, :])
                outsb = headp.tile([items, K], f32)
                for jc in range(K // 512):
                    ph = ps_h.tile([items, 512], f32, tag="ph", name="ph")
                    for ct in range(CT):
                        hw_t = headp.tile([128, 512], f32r, tag="hw", bufs=4,
                                          name="hw_t")
                        nc.sync.dma_start(
                            out=hw_t, in_=headwg[ct, :, jc * 512:(jc + 1) * 512])
                        nc.tensor.matmul(ph, xmall[:, ct, :items], hw_t,
                                         start=(ct == 0), stop=False)
                    nc.tensor.matmul(ph, ones8, hb_t[:, jc * 512:(jc + 1) * 512],
                                     start=False, stop=True)
                    nc.scalar.activation(out=outsb[:, jc * 512:(jc + 1) * 512],
                                         in_=ph, func=AF.Copy)
                nc.sync.dma_start(out=out[:, :], in_=outsb)

    nc.compile()
    return nc


# ---------------------------------------------------------------------------
# host-side preprocessing
# ---------------------------------------------------------------------------

def prep_inputs(inputs, stem_w, stem_b, ln1_g, ln1_b, tok_w1, tok_b1, tok_w2,
                tok_b2, ln2_g, ln2_b, ch_w1, ch_b1, ch_w2, ch_b2, lnf_g, lnf_b,
                head_w, head_b, items=IPC, blocks=L):
    """Returns (shared_map, per_core_list, flags)."""
    f = np.float32
    inputs = np.asarray(inputs, f)
    # patches: (B, CIN, 16, 2, 16, 2) -> (B, n=256, q=8); +ones row -> (B,9,256)
    x = inputs.reshape(B, CIN, H // 2, 2, W // 2, 2).transpose(0, 2, 4, 1, 3, 5)
    x = x.reshape(B, N, CIN * 4)
    ptA = np.concatenate([x.transpose(0, 2, 1),
                          np.ones((B, 1, N), f)], axis=1)  # (B, 9, 256)

    wq = np.concatenate([np.asarray(stem_w, f).reshape(C, 8).T,
                         np.asarray(stem_b, f)[None, :]], axis=0)  # (9, C)

    blocks = max(blocks, 1)
    w1cum = np.cumsum(np.asarray(tok_w1, f), axis=1)[:blocks]        # (L, N, TOK)
    tokw1c = np.ascontiguousarray(w1cum.reshape(blocks, NT, 128, TOK))
    tokw2 = np.ascontiguousarray(np.asarray(tok_w2, f)[:blocks]
                                 .reshape(blocks, TT, 128, N))
    tokb1 = np.ascontiguousarray(np.asarray(tok_b1, f)[:blocks]
                                 .reshape(blocks, TT, 128).transpose(0, 2, 1))

    g2 = np.asarray(ln2_g, f)[:blocks]
    b2 = np.asarray(ln2_b, f)[:blocks]
    cw1 = np.asarray(ch_w1, f)[:blocks]
    w1g_full = g2[:, :, None] * cw1                                   # (L, C, CH)
    w1g = (w1g_full.reshape(blocks, CT, 128, MT, 128)
           .transpose(0, 3, 2, 1, 4)                 # (L, MT, 128, CT, 128)
           .reshape(blocks, MT // 2, 2, 128, CT, 128)
           .transpose(0, 1, 3, 2, 4, 5))             # (L, 12, 128, 2, CT, 128)
    w1g = np.ascontiguousarray(w1g)
    v = np.einsum("lc,lcm->lm", b2, cw1) + np.asarray(ch_b1, f)[:blocks]
    vb1 = np.ascontiguousarray(v.reshape(blocks, MT, 128).transpose(0, 2, 1))
    chw2 = (np.asarray(ch_w2, f)[:blocks]
            .reshape(blocks, MT // 2, 2, 128, C)
            .transpose(0, 1, 3, 2, 4))               # (L, 12, 128, 2, C)
    chw2 = np.ascontiguousarray(chw2)
    chb2c = np.ascontiguousarray(np.asarray(ch_b2, f)[:blocks]
                                 .reshape(blocks, CT, 128).transpose(0, 2, 1))

    gf = np.asarray(lnf_g, f)
    bf = np.asarray(lnf_b, f)
    hw = np.asarray(head_w, f)
    headwg = np.ascontiguousarray((gf[:, None] * hw).reshape(CT, 128, K))
    headb = (bf @ hw + np.asarray(head_b, f)).reshape(1, K).astype(f)

    ln1g = np.ascontiguousarray(np.asarray(ln1_g, f)[:blocks])
    ln1b = np.ascontiguousarray(np.asarray(ln1_b, f)[:blocks])
    has_g1 = not np.all(ln1g == 1.0)
    has_b1 = not np.all(ln1b == 0.0)

    shared = dict(wq=wq, tokw1c=tokw1c, tokw2=tokw2, tokb1=tokb1, w1g=w1g,
                  vb1=vb1, chw2=chw2, chb2c=chb2c, headwg=headwg, headb=headb,
                  ln1g=ln1g, ln1b=ln1b)
    shared = {k: np.ascontiguousarray(v, f) for k, v in shared.items()}

    per_core = []
    for c in range(NCORES):
        sel = ptA[c * IPC:(c + 1) * IPC][:items]  # (items, 9, 256)
        ptc = np.ascontiguousarray(sel.transpose(1, 0, 2).reshape(9, items * N))
        per_core.append(dict(pt=ptc))
    return shared, per_core, dict(has_g1=has_g1, has_b1=has_b1)


_CACHE = {}


def kernel(**inputs):
    from concourse.bass_utils import run_bass_kernel_spmd
    shared, per_core, flags = prep_inputs(**inputs)
    key = (flags["has_g1"], flags["has_b1"])
    if key not in _CACHE:
        _CACHE[key] = build(has_g1=flags["has_g1"], has_b1=flags["has_b1"])
    nc = _CACHE[key]
    in_maps = [{**shared, **pc} for pc in per_core]
    res = run_bass_kernel_spmd(nc, in_maps, core_ids=list(range(NCORES)))
    outs = [r["out"] for r in res.results]
    return np.concatenate(outs, axis=0).astype(np.float32)



# revision 40
# speedup vs baseline: 1.2206x; 1.0002x over previous
"""AutoregressiveMlpMixer forward on 8 Trainium2 NeuronCores (Bass/Tile).

Strategy
- Pure data parallelism: 64 batch items -> 8 per core, weights replicated.
- The reverse cumsum over tokens is folded into tok_w1 on the host
  (suffix-sum then matmul == matmul with prefix-cumsum'd weights).
- LN2 / final-LN affine params are folded into the following matmul weights
  on the host. tok_b2 is dropped exactly (it is constant along the LN2
  normalization axis, so LN2 cancels it).
- Inter-block state X is kept TRANSPOSED ([channel, token] tiles): the
  channel-MLP second matmul then accumulates its 24 k-tiles into 6
  persistent PSUM banks while E/F stream weights fused per m-tile, so the
  gelu intermediate never materializes. LN1 re-transposes X on the PE.
- Matmuls run in float32r (~13 mantissa bits, full PE rate at >=256 moving
  rows). fp8/bf16 matmuls are NOT usable: this no-residual network
  amplifies operand quantization noise ~25x (all-bf16 -> 1.7e-2 final rel
  err, fp8 channel-MLP -> 2e-1). Storage-only bf16 (X / xn / zn tiles, PE
  transposes at 1.0 cyc/row) costs ~7e-3 total and stays within budget.
- Per-engine instruction streams are in-order, so cross-engine latency
  chains are hidden by a 4-phase software pipeline emitted per step s:
  DZ(s) (LN2 transposes, inputs an iter old), A(s+2) (LN1 stats + batched
  DVE-Newton rsqrt + apply), BC(s+1) (token-MLP + LN2 stats/apply), EF(s)
  (channel-MLP, E,E,F,F per weight pair so each gelu hides under the other
  matmul). The DVE Newton rsqrt (batched over 4 LN sites) keeps
  Abs_reciprocal_sqrt off the ACT engine: no 1283ns act-table reloads.
- Channel-MLP weights stream once per G=2 items as paired-m-tile DMAs.
"""

import sys

sys.path.insert(0, "/opt/trn_rl_repo")

import numpy as np

import concourse.bass as bass
import concourse.tile as tile
from concourse import bacc, masks, mybir

f32 = mybir.dt.float32
f32r = mybir.dt.float32r
AF = mybir.ActivationFunctionType
ALU = mybir.AluOpType

# Model dims (hardcoded per problem spec)
B, CIN, H, W = 64, 2, 32, 32
N = 256          # tokens
C = 768          # hidden dim
TOK = 512        # tokens_mlp_dim
CH = 3072        # channels_mlp_dim
L = 8            # blocks
K = 2048         # classes
EPS = 1e-5

NCORES = 8
IPC = B // NCORES    # items per core = 8
NT = N // 128        # 2 token tiles per item
CT = C // 128        # 6 channel tiles
MT = CH // 128       # 24 channel-mlp tiles
TT = TOK // 128      # 4 token-mlp tiles
CC = (512, 256)      # channel free-dim chunks for 768
CCO = (0, 512)
G = 2                # items per channel-MLP weight pass


def _ln_finish(nc, pool, st, magic_t, mode="dve"):
    """bn_aggr + rsqrt. st: [128, s, 6] bn_stats. Returns (mu, rstd) APs."""
    return _ln_finish_batch(nc, pool, [st], magic_t, mode)[0]


def _ln_finish_batch(nc, pool, sts, magic_t, mode="dve"):
    """Batched bn_aggr + Newton rsqrt for n<=4 LN sites on the DVE.

    sts: list of [128, 3, 6] bn_stats tiles. Returns [(mu, rstd)] col APs.
    Batching amortizes the per-op overhead of the 9-op Newton chain."""
    i32 = mybir.dt.int32
    n = len(sts)
    # [128, 2, n]: row 0 = means, row 1 = vars -> var row is CONTIGUOUS so
    # the bitcast in the Newton iteration below is legal.
    mv = pool.tile([128, 2, n], f32, tag=f"ln_mv{n}", bufs=4, name="mv")
    for s, st in enumerate(sts):
        nc.vector.bn_aggr(out=mv[:, :, s], in_=st)
    v = mv[:, 1, :]
    if mode == "act":
        nc.scalar.activation(out=v, in_=v, func=AF.Abs_reciprocal_sqrt,
                             bias=magic_t[1], scale=1.0)
        return [(mv[:, 0, s:s + 1], v[:, s:s + 1]) for s in range(n)]
    eng = nc.gpsimd if mode == "pool" else nc.vector
    eng.tensor_scalar_add(v, v, float(EPS))
    iv = pool.tile([128, n], i32, tag=f"rs_i{n}", bufs=4, name="iv")
    eng.tensor_scalar(iv, v.bitcast(i32), 1, None,
                      ALU.logical_shift_right)
    eng.tensor_tensor(iv, magic_t[0][:, :n], iv, ALU.subtract)
    y = iv.bitcast(f32)
    t = pool.tile([128, n], f32, tag=f"rs_t{n}", bufs=4, name="t")
    for _ in range(3):
        eng.tensor_mul(t, y, y)
        eng.tensor_mul(t, t, v)
        eng.tensor_scalar(t, t, -0.5, 1.5, ALU.mult, ALU.add)
        eng.tensor_mul(y, y, t)
    return [(mv[:, 0, s:s + 1], y[:, s:s + 1]) for s in range(n)]


def _ln_stats(nc, pool, x, magic_t, mode="dve"):
    """mean/rstd of x[128, C] over the free dim. Returns (mu, rstd) col APs."""
    st = pool.tile([128, 3, 6], f32, tag="ln_st", bufs=12, name="st")
    xg = x.rearrange("p (s q) -> p s q", s=3)
    for s in range(3):
        nc.vector.bn_stats(out=st[:, s, :], in_=xg[:, s, :])
    return _ln_finish(nc, pool, st, magic_t, mode)


def build(items=IPC, blocks=L, has_g1=False, has_b1=False, kchunk=24,
          rsqrt="dve", pipelined=True):
    """Build the SPMD program for one core processing `items` batch items."""
    nc = bacc.Bacc("TRN2", target_bir_lowering=False, debug=False)

    # ---- DRAM tensors (names = in_map keys) ----
    pt = nc.dram_tensor("pt", [9, items * N], f32r, kind="ExternalInput")
    wq = nc.dram_tensor("wq", [9, C], f32r, kind="ExternalInput")
    bl = max(blocks, 1)
    tokw1c = nc.dram_tensor("tokw1c", [bl, NT, 128, TOK], f32r, kind="ExternalInput")
    tokw2 = nc.dram_tensor("tokw2", [bl, TT, 128, N], f32r, kind="ExternalInput")
    tokb1 = nc.dram_tensor("tokb1", [bl, 128, TT], f32, kind="ExternalInput")
    w1g = nc.dram_tensor("w1g", [bl, MT // 2, 128, 2, CT, 128], f32r,
                         kind="ExternalInput")
    vb1 = nc.dram_tensor("vb1", [bl, 128, MT], f32, kind="ExternalInput")
    chw2 = nc.dram_tensor("chw2", [bl, MT // 2, 128, 2, C], f32r,
                          kind="ExternalInput")
    chb2c = nc.dram_tensor("chb2c", [bl, 128, CT], f32, kind="ExternalInput")
    headwg = nc.dram_tensor("headwg", [CT, 128, K], f32r, kind="ExternalInput")
    ln1g = nc.dram_tensor("ln1g", [bl, C], f32, kind="ExternalInput")
    ln1b = nc.dram_tensor("ln1b", [bl, C], f32, kind="ExternalInput")
    out = nc.dram_tensor("out", [items, K], f32, kind="ExternalOutput")

    n_groups = (items + G - 1) // G

    with tile.TileContext(nc) as tc:
        with tc.tile_pool(name="const", bufs=1) as const, \
             tc.tile_pool(name="xstate", bufs=1) as xstate:
            magic_i = const.tile([128, 4], mybir.dt.int32, name="magic_i")
            nc.vector.memset(magic_i, 0x5F3759DF)
            eps_col = const.tile([128, 1], f32, name="eps_col")
            nc.vector.memset(eps_col, EPS)
            magic_t = (magic_i, eps_col)
            ident = const.tile([128, 128], f32, name="ident")
            masks.make_identity(nc, ident)
            identr = const.tile([128, 128], f32r, name="identr")
            nc.vector.tensor_copy(identr, ident)
            bf16 = mybir.dt.bfloat16
            identb = const.tile([128, 128], bf16, name="identb")
            nc.vector.tensor_copy(identb, ident)

            # persistent state, TRANSPOSED: X[item][ct] = [128(c), N(tokens)]
            # bf16: PE transposes run at 1.0 cyc/row and SBUF halves (the
            # ~0.2% storage noise costs ~6e-3 final rel err, within budget)
            X = [[xstate.tile([128, N], bf16, tag=f"x_{i}_{ct}",
                              name=f"x_{i}_{ct}")
                  for ct in range(CT)] for i in range(items)]

            # ---------------- mixer blocks (stem + head share the scope so
            # no pool-release barrier serializes the phase boundaries) ------
            with tc.tile_pool(name="tokw", bufs=2) as tokwp, \
                 tc.tile_pool(name="lnp", bufs=4) as lnp, \
                 tc.tile_pool(name="acts", bufs=1) as acts, \
                 tc.tile_pool(name="wstream", bufs=2) as wstream, \
                 tc.tile_pool(name="ps_mm", bufs=8, space="PSUM") as ps_mm:

                # ---- stem (writes X transposed) ----
                ptt = acts.tile([9, items * N], f32r, tag="ptt", name="ptt")
                nc.sync.dma_start(out=ptt, in_=pt[:, :])
                wqt = acts.tile([9, C], f32r, tag="wqt", name="wqt")
                nc.sync.dma_start(out=wqt, in_=wq[:, :])
                # prefetch the first channel-MLP weight pairs so block 0's
                # E matmuls don't wait on DMAs queued behind the stem
                wpf = {}
                if blocks > 0:
                    for pp in (0, 1):
                        w1g_t = wstream.tile([128, 2, CT, 128], f32r,
                                             tag="w1g", name="w1g_t")
                        nc.sync.dma_start(out=w1g_t, in_=w1g[0, pp])
                        w2c_t = wstream.tile([128, 2, C], f32r,
                                             tag="w2c", name="w2c_t")
                        nc.sync.dma_start(out=w2c_t, in_=chw2[0, pp])
                        wpf[(0, pp)] = (w1g_t, w2c_t)
                nw_all = items * N
                nchunks = [(o, min(512, nw_all - o)) for o in range(0, nw_all, 512)]
                for ct in range(CT):
                    for (no, nn) in nchunks:
                        pss = ps_mm.tile([128, 512], f32, tag="mm", name="pss")
                        nc.tensor.matmul(pss[:, :nn],
                                         wqt[:, ct * 128:(ct + 1) * 128],
                                         ptt[:, no:no + nn],
                                         start=True, stop=True)
                        for j in range(0, nn, N):
                            i = (no + j) // N
                            nc.scalar.activation(out=X[i][ct],
                                                 in_=pss[:, j:j + N],
                                                 func=AF.Copy)

                blk_w = {}

                def emit_tok_weights(l):
                    w = {}
                    w1c_t = tokwp.tile([128, NT, TOK], f32r, tag="w1c",
                                       name="w1c")
                    nc.sync.dma_start(out=w1c_t,
                                      in_=tokw1c[l].rearrange("k p t -> p k t"))
                    w2_t = tokwp.tile([128, TT, N], f32r, tag="w2", name="w2")
                    nc.sync.dma_start(out=w2_t,
                                      in_=tokw2[l].rearrange("k p n -> p k n"))
                    b1_t = tokwp.tile([128, TT], f32, tag="b1", name="b1")
                    nc.sync.dma_start(out=b1_t, in_=tokb1[l])
                    vb1_t = tokwp.tile([128, MT], f32, tag="vb1", name="vb1")
                    nc.sync.dma_start(out=vb1_t, in_=vb1[l])
                    chb2_t = tokwp.tile([128, CT], f32, tag="chb2", name="chb2")
                    nc.sync.dma_start(out=chb2_t, in_=chb2c[l])
                    w.update(w1c=w1c_t, w2=w2_t, b1=b1_t, vb1=vb1_t,
                             chb2=chb2_t)
                    if has_g1:
                        g1_t = tokwp.tile([128, C], f32, tag="g1", name="g1")
                        nc.sync.dma_start(
                            out=g1_t,
                            in_=ln1g.ap()[l:l + 1, :].partition_broadcast(128))
                        w["g1"] = g1_t
                    if has_b1:
                        b1v_t = tokwp.tile([128, C], f32, tag="b1v", name="b1v")
                        nc.sync.dma_start(
                            out=b1v_t,
                            in_=ln1b.ap()[l:l + 1, :].partition_broadcast(128))
                        w["b1v"] = b1v_t
                    return w

                def emit_A(l, g):
                    """LN1 for group g: transposes + stats + batched Newton
                    + apply -> Y tiles. Emitted 2 steps ahead of its EF so
                    the DVE Newton chain never gates the PE stream."""
                    if l not in blk_w:
                        blk_w[l] = emit_tok_weights(l)
                    g1_t = blk_w[l].get("g1")
                    b1v_t = blk_w[l].get("b1v")
                    gitems = list(range(g * G, min((g + 1) * G, items)))
                    pre = []
                    sts = []
                    for i2, i in enumerate(gitems):
                        xn = [lnp.tile([128, C], bf16, tag="xn", bufs=4,
                                       name="xn") for _ in range(NT)]
                        for t in range(NT):
                            st = lnp.tile([128, 3, 6], f32, tag="ln_st",
                                          bufs=12, name="st")
                            for cg, cn in ((0, 4), (4, 2)):
                                ptr = ps_mm.tile([128, cn * 128], bf16,
                                                 tag="mm", name="ptrA")
                                for cc in range(cn):
                                    nc.tensor.transpose(
                                        ptr[:, cc * 128:(cc + 1) * 128],
                                        X[i][cg + cc][:, t * 128:(t + 1) * 128],
                                        identb)
                                nc.scalar.activation(
                                    out=xn[t][:, cg * 128:(cg + cn) * 128],
                                    in_=ptr, func=AF.Copy)
                                pgg = ptr.rearrange("p (s q) -> p s q", q=256)
                                for s in range(cn // 2):
                                    nc.vector.bn_stats(
                                        out=st[:, cg // 2 + s, :],
                                        in_=pgg[:, s, :])
                            sts.append(st)
                        pre.append(xn)
                    musall = _ln_finish_batch(nc, lnp, sts, magic_t, rsqrt)
                    Ys = []
                    for i2, i in enumerate(gitems):
                        xn = pre[i2]
                        Y = []
                        for t in range(NT):
                            mu, rstd = musall[i2 * NT + t]
                            yt = lnp.tile([128, C], f32r, tag="y", bufs=8,
                                          name="yt")
                            for cw, co in zip(CC, CCO):
                                nc.vector.tensor_scalar(
                                    out=yt[:, co:co + cw],
                                    in0=xn[t][:, co:co + cw],
                                    scalar1=mu, scalar2=rstd,
                                    op0=ALU.subtract, op1=ALU.mult)
                            if has_g1:
                                nc.vector.tensor_mul(yt, yt, g1_t)
                            if has_b1:
                                nc.vector.tensor_add(yt, yt, b1v_t)
                            Y.append(yt)
                        Ys.append(Y)
                    return Ys

                def emit_BC(l, g, Ys):
                    """token-mix + LN2 stats for group g -> (y2 tiles, rstds).
                    The LN2 apply/transpose (emit_DZ) is emitted an iteration
                    later so its Newton chain never gates the PE stream."""
                    w1c_t, w2_t, b1_t = (blk_w[l][k] for k in ("w1c", "w2", "b1"))
                    gitems = list(range(g * G, min((g + 1) * G, items)))
                    out = []
                    y1s = []
                    # ---- B for ALL items first: the last B-gelu's latency
                    # then hides under the other item's C matmuls ----
                    for i2, i in enumerate(gitems):
                        Y = Ys[i2]
                        y1 = []
                        for mt in range(TT):
                            yg = lnp.tile([128, C], f32r, tag="y1g", bufs=8,
                                          name="yg")
                            for ci, (cw, co) in enumerate(zip(CC, CCO)):
                                pb = ps_mm.tile([128, 512], f32, tag="mm",
                                                name="pb")
                                for k in range(NT):
                                    nc.tensor.matmul(
                                        pb[:, :cw],
                                        w1c_t[:, k, mt * 128:(mt + 1) * 128],
                                        Y[k][:, co:co + cw],
                                        start=(k == 0), stop=(k == NT - 1))
                                nc.scalar.activation(
                                    out=yg[:, co:co + cw], in_=pb[:, :cw],
                                    func=AF.Gelu, bias=b1_t[:, mt:mt + 1],
                                    scale=1.0)
                            y1.append(yg)
                        y1s.append(y1)
                    for i2, i in enumerate(gitems):
                        y1 = y1s[i2]
                        # ---- C: y2 = w2^T @ y1, stats from PSUM ----
                        cpost = []
                        csts = []
                        for t in range(NT):
                            y2t = lnp.tile([128, C], f32, tag="y2", bufs=4,
                                           name="y2t")
                            st = lnp.tile([128, 3, 6], f32, tag="ln_st",
                                          bufs=12, name="st")
                            for ci, (cw, co) in enumerate(zip(CC, CCO)):
                                pc = ps_mm.tile([128, 512], f32, tag="mm",
                                                name="pc")
                                for k in range(TT):
                                    nc.tensor.matmul(
                                        pc[:, :cw],
                                        w2_t[:, k, t * 128:(t + 1) * 128],
                                        y1[k][:, co:co + cw],
                                        start=(k == 0), stop=(k == TT - 1))
                                nc.scalar.activation(out=y2t[:, co:co + cw],
                                                     in_=pc[:, :cw],
                                                     func=AF.Copy)
                                # LN2 stats straight from PSUM
                                pg = pc[:, :cw].rearrange(
                                    "p (s q) -> p s q", q=256)
                                for s in range(cw // 256):
                                    nc.vector.bn_stats(
                                        out=st[:, 2 * ci + s, :],
                                        in_=pg[:, s, :])
                            cpost.append(y2t)
                            csts.append(st)
                        cmus = _ln_finish_batch(nc, lnp, csts, magic_t, rsqrt)
                        # LN2 apply now (its Newton latency hides under the
                        # concurrent EF); the transposes wait for emit_DZ.
                        zns = []
                        for t in range(NT):
                            mu, rstd = cmus[t]
                            zn = lnp.tile([128, C], bf16, tag="z", bufs=8,
                                          name="zn")
                            for cw, co in zip(CC, CCO):
                                nc.vector.tensor_scalar(
                                    out=zn[:, co:co + cw],
                                    in0=cpost[t][:, co:co + cw],
                                    scalar1=mu, scalar2=rstd,
                                    op0=ALU.subtract, op1=ALU.mult)
                            zns.append(zn)
                        out.append(zns)
                    return out

                def emit_DZ(l, g, bc):
                    """Transpose LN2 output into Zt. Emitted FIRST in its
                    iteration, one after emit_BC: the zn tiles are ready, so
                    the PE transposes and DVE copies fire immediately."""
                    Zt = acts.tile([128, CT, G * N], f32r, tag="zt",
                                   bufs=2, name="zt")
                    for i2, zns in enumerate(bc):
                        for t in range(NT):
                            zn = zns[t]
                            for cg, cn in ((0, 4), (4, 2)):
                                ptr = ps_mm.tile([128, cn * 128], bf16,
                                                 tag="mm", name="ptrT")
                                for cc in range(cn):
                                    nc.tensor.transpose(
                                        ptr[:, cc * 128:(cc + 1) * 128],
                                        zn[:, (cg + cc) * 128:
                                           (cg + cc + 1) * 128],
                                        identb)
                                nc.vector.tensor_copy(
                                    Zt[:, cg:cg + cn,
                                       i2 * N + t * 128:i2 * N + (t + 1) * 128],
                                    ptr.rearrange("p (c q) -> p c q", q=128))
                    return Zt

                def emit_EF(l, g, Zt, kchunk=kchunk):
                    """fused channel-MLP over m-tiles for group g of block l.

                    F accumulates in PSUM per k-chunk, then folds into the
                    SBUF state X (copy w/ bias on chunk 0, add afterwards) so
                    PSUM banks are only held transiently.
                    """
                    vb1_t = blk_w[l]["vb1"]
                    chb2_t = blk_w[l]["chb2"]
                    gitems = list(range(g * G, min((g + 1) * G, items)))
                    nw = len(gitems) * N

                    for k0 in range(0, MT, kchunk):
                        psF = [ps_mm.tile([128, G * N], f32, tag="mm",
                                          name=f"pf_{ct}") for ct in range(CT)]
                        for pp in range(k0 // 2, (k0 + kchunk) // 2):
                            if (l, pp) in wpf:
                                w1g_t, w2c_t = wpf.pop((l, pp))
                            else:
                                w1g_t = wstream.tile([128, 2, CT, 128], f32r,
                                                     tag="w1g", name="w1g_t")
                                nc.sync.dma_start(out=w1g_t, in_=w1g[l, pp])
                                w2c_t = wstream.tile([128, 2, C], f32r,
                                                     tag="w2c", name="w2c_t")
                                nc.sync.dma_start(out=w2c_t, in_=chw2[l, pp])
                            # E,E then F,F per pair: each gelu's latency
                            # hides under the other matmul of the pair.
                            hgs = []
                            for j in (0, 1):
                                mt = 2 * pp + j
                                pe = ps_mm.tile([128, 512], f32, tag="mm",
                                                name="pe")
                                for kc in range(CT):
                                    nc.tensor.matmul(pe[:, :nw],
                                                     w1g_t[:, j, kc, :],
                                                     Zt[:, kc, :nw],
                                                     start=(kc == 0),
                                                     stop=(kc == CT - 1))
                                hg_cur = acts.tile([128, G * N], f32r,
                                                   tag="hg", bufs=3,
                                                   name="hg")
                                nc.scalar.activation(out=hg_cur[:, :nw],
                                                     in_=pe[:, :nw],
                                                     func=AF.Gelu,
                                                     bias=vb1_t[:, mt:mt + 1],
                                                     scale=1.0)
                                hgs.append(hg_cur)
                            for j in (0, 1):
                                mt = 2 * pp + j
                                for ct in range(CT):
                                    nc.tensor.matmul(
                                        psF[ct][:, :nw],
                                        w2c_t[:, j, ct * 128:(ct + 1) * 128],
                                        hgs[j][:, :nw],
                                        start=(mt == k0),
                                        stop=(mt == k0 + kchunk - 1))
                        for ct in range(CT):
                            for i2, i in enumerate(gitems):
                                src = psF[ct][:, i2 * N:(i2 + 1) * N]
                                if k0 == 0:
                                    nc.scalar.activation(
                                        out=X[i][ct], in_=src,
                                        func=AF.Identity,
                                        bias=chb2_t[:, ct:ct + 1], scale=1.0)
                                else:
                                    nc.vector.tensor_add(X[i][ct], X[i][ct],
                                                         src)

                # ---- head helpers (emitted per group after its last EF so
                # the final-LN work overlaps the remaining groups' EF) ----
                invn_f = acts.tile([128, 2], f32, tag="invnf", name="invn_f")
                nc.vector.memset(invn_f, 1.0 / N)
                invn_col = acts.tile([128, 2], f32r, tag="invn", name="invn")
                nc.vector.tensor_copy(invn_col, invn_f)
                xmall = acts.tile([128, CT, items], f32r, tag="xmall",
                                  name="xmall")

                def emit_head_pre(g):
                    """Final-LN transposes + stats + batched Newton for a
                    group; the apply/mean (emit_head_post) follows one EF
                    later so the rstd chain never gates the PE stream."""
                    gitems = list(range(g * G, min((g + 1) * G, items)))
                    xfs, sts = [], []
                    for i in gitems:
                        xf = [lnp.tile([128, C], bf16, tag="xf", bufs=4,
                                       name="xf") for _ in range(NT)]
                        for ct in range(CT):
                            for t in range(NT):
                                ptr = ps_mm.tile([128, 128], bf16, tag="mm",
                                                 name="ptrH")
                                nc.tensor.transpose(
                                    ptr, X[i][ct][:, t * 128:(t + 1) * 128],
                                    identb)
                                nc.vector.tensor_copy(
                                    xf[t][:, ct * 128:(ct + 1) * 128], ptr)
                        for t in range(NT):
                            st = lnp.tile([128, 3, 6], f32, tag="ln_st",
                                          bufs=12, name="st")
                            xg = xf[t].rearrange("p (s q) -> p s q", s=3)
                            for s in range(3):
                                nc.vector.bn_stats(out=st[:, s, :],
                                                   in_=xg[:, s, :])
                            sts.append(st)
                        xfs.append(xf)
                    mus = _ln_finish_batch(nc, lnp, sts, magic_t, rsqrt)
                    return (gitems, xfs, mus)

                def emit_head_post(hp):
                    gitems, xfs, mus = hp
                    for i2, i in enumerate(gitems):
                        xh = []
                        for t in range(NT):
                            mu, rstd = mus[i2 * NT + t]
                            xht = lnp.tile([128, C], f32r, tag="xh", bufs=2,
                                           name="xht")
                            nc.vector.tensor_scalar(
                                out=xht, in0=xfs[i2][t], scalar1=mu,
                                scalar2=rstd,
                                op0=ALU.subtract, op1=ALU.mult)
                            xh.append(xht)
                        for ct in range(CT):
                            pxm = ps_mm.tile([128, 2], f32, tag="mm",
                                             name="pxm")
                            for t in range(NT):
                                nc.tensor.matmul(
                                    pxm, xh[t][:, ct * 128:(ct + 1) * 128],
                                    invn_col,
                                    start=(t == 0), stop=(t == NT - 1))
                            nc.scalar.activation(out=xmall[:, ct, i:i + 1],
                                                 in_=pxm[:, 0:1], func=AF.Copy)

                def emit_head_final():
                    outsb = acts.tile([items, K], f32, tag="ptt",
                                      name="outsb")
                    for jc in range(K // 512):
                        ph = ps_mm.tile([items, 512], f32, tag="mm", name="ph")
                        for ct in range(CT):
                            hw_t = acts.tile([128, 512], f32r, tag="hw",
                                             bufs=4, name="hw_t")
                            nc.sync.dma_start(
                                out=hw_t,
                                in_=headwg[ct, :, jc * 512:(jc + 1) * 512])
                            nc.tensor.matmul(ph, xmall[:, ct, :items], hw_t,
                                             start=(ct == 0),
                                             stop=(ct == CT - 1))
                        nc.scalar.activation(
                            out=outsb[:, jc * 512:(jc + 1) * 512],
                            in_=ph, func=AF.Copy)
                    nc.sync.dma_start(out=out[:, :], in_=outsb)

                # software-pipelined emission: per-engine instruction streams
                # are in-order, so A (whose LN Newton chain gates B) is
                # emitted TWO steps ahead and BCD one step ahead of EF.
                seq = [(l, g) for l in range(blocks) for g in range(n_groups)]
                ys, bcs, hps = {}, {}, []
                # A(s+la) reads X written by EF(s+la-n_groups), so the A
                # lookahead must stay below n_groups (program order = dep
                # order for the tile framework). Per iteration the emission
                # order is A(s+2), BC(s+1), DZ(s), EF(s): every cross-engine
                # latency chain (LN Newton on DVE) resolves a full EF phase
                # before the PE stream needs its result.
                la = min(2, n_groups - 1) if pipelined else 0
                lb = min(1, la)
                if la > 0:
                    for k in range(min(la, len(seq))):
                        ys[seq[k]] = emit_A(*seq[k])
                    for k in range(min(lb, len(seq))):
                        bcs[seq[k]] = emit_BC(*seq[k], ys.pop(seq[k]))
                    for idx, key in enumerate(seq):
                        zt = emit_DZ(*key, bcs[key])
                        if idx + la < len(seq):
                            nkey = seq[idx + la]
                            ys[nkey] = emit_A(*nkey)
                        if idx + lb < len(seq):
                            nkey = seq[idx + lb]
                            if nkey not in bcs:
                                bcs[nkey] = emit_BC(*nkey, ys.pop(nkey))
                        bcs.pop(key)
                        emit_EF(*key, zt)
                        if key[0] == blocks - 1:
                            if hps:
                                emit_head_post(hps.pop(0))
                            hps.append(emit_head_pre(key[1]))
                else:
                    for key in seq:
                        emit_EF(*key, emit_DZ(*key, emit_BC(*key,
                                                            emit_A(*key))))
                        if key[0] == blocks - 1:
                            if hps:
                                emit_head_post(hps.pop(0))
                            hps.append(emit_head_pre(key[1]))
                if not seq:
                    for g in range(n_groups):
                        hps.append(emit_head_pre(g))
                while hps:
                    emit_head_post(hps.pop(0))
                emit_head_final()

    nc.compile()
    return nc


# ---------------------------------------------------------------------------
# host-side preprocessing
# ---------------------------------------------------------------------------

def prep_inputs(inputs, stem_w, stem_b, ln1_g, ln1_b, tok_w1, tok_b1, tok_w2,
                tok_b2, ln2_g, ln2_b, ch_w1, ch_b1, ch_w2, ch_b2, lnf_g, lnf_b,
                head_w, head_b, items=IPC, blocks=L):
    """Returns (shared_map, per_core_list, flags)."""
    f = np.float32
    inputs = np.asarray(inputs, f)
    # patches: (B, CIN, 16, 2, 16, 2) -> (B, n=256, q=8); +ones row -> (B,9,256)
    x = inputs.reshape(B, CIN, H // 2, 2, W // 2, 2).transpose(0, 2, 4, 1, 3, 5)
    x = x.reshape(B, N, CIN * 4)
    ptA = np.concatenate([x.transpose(0, 2, 1),
                          np.ones((B, 1, N), f)], axis=1)  # (B, 9, 256)

    wq = np.concatenate([np.asarray(stem_w, f).reshape(C, 8).T,
                         np.asarray(stem_b, f)[None, :]], axis=0)  # (9, C)

    blocks = max(blocks, 1)
    w1cum = np.cumsum(np.asarray(tok_w1, f), axis=1)[:blocks]        # (L, N, TOK)
    tokw1c = np.ascontiguousarray(w1cum.reshape(blocks, NT, 128, TOK))
    tokw2 = np.ascontiguousarray(np.asarray(tok_w2, f)[:blocks]
                                 .reshape(blocks, TT, 128, N))
    tokb1 = np.ascontiguousarray(np.asarray(tok_b1, f)[:blocks]
                                 .reshape(blocks, TT, 128).transpose(0, 2, 1))

    g2 = np.asarray(ln2_g, f)[:blocks]
    b2 = np.asarray(ln2_b, f)[:blocks]
    cw1 = np.asarray(ch_w1, f)[:blocks]
    w1g_full = g2[:, :, None] * cw1                                   # (L, C, CH)
    w1g = (w1g_full.reshape(blocks, CT, 128, MT, 128)
           .transpose(0, 3, 2, 1, 4)                 # (L, MT, 128, CT, 128)
           .reshape(blocks, MT // 2, 2, 128, CT, 128)
           .transpose(0, 1, 3, 2, 4, 5))             # (L, 12, 128, 2, CT, 128)
    w1g = np.ascontiguousarray(w1g)
    v = np.einsum("lc,lcm->lm", b2, cw1) + np.asarray(ch_b1, f)[:blocks]
    vb1 = np.ascontiguousarray(v.reshape(blocks, MT, 128).transpose(0, 2, 1))
    chw2 = (np.asarray(ch_w2, f)[:blocks]
            .reshape(blocks, MT // 2, 2, 128, C)
            .transpose(0, 1, 3, 2, 4))               # (L, 12, 128, 2, C)
    chw2 = np.ascontiguousarray(chw2)
    chb2c = np.ascontiguousarray(np.asarray(ch_b2, f)[:blocks]
                                 .reshape(blocks, CT, 128).transpose(0, 2, 1))

    gf = np.asarray(lnf_g, f)
    bf = np.asarray(lnf_b, f)
    hw = np.asarray(head_w, f)
    headwg = np.ascontiguousarray((gf[:, None] * hw).reshape(CT, 128, K))
    headb = (bf @ hw + np.asarray(head_b, f)).reshape(1, K).astype(f)

    ln1g = np.ascontiguousarray(np.asarray(ln1_g, f)[:blocks])
    ln1b = np.ascontiguousarray(np.asarray(ln1_b, f)[:blocks])
    has_g1 = not np.all(ln1g == 1.0)
    has_b1 = not np.all(ln1b == 0.0)

    shared = dict(wq=wq, tokw1c=tokw1c, tokw2=tokw2, tokb1=tokb1, w1g=w1g,
                  vb1=vb1, chw2=chw2, chb2c=chb2c, headwg=headwg, headb=headb,
                  ln1g=ln1g, ln1b=ln1b)
    shared = {k: np.ascontiguousarray(v, f) for k, v in shared.items()}

    per_core = []
    for c in range(NCORES):
        sel = ptA[c * IPC:(c + 1) * IPC][:items]  # (items, 9, 256)
        ptc = np.ascontiguousarray(sel.transpose(1, 0, 2).reshape(9, items * N))
        per_core.append(dict(pt=ptc))
    return shared, per_core, dict(has_g1=has_g1, has_b1=has_b1)


_CACHE = {}


def kernel(**inputs):
    from concourse.bass_utils import run_bass_kernel_spmd
    shared, per_core, flags = prep_inputs(**inputs)
    key = (flags["has_g1"], flags["has_b1"])
    if key not in _CACHE:
        _CACHE[key] = build(has_g1=flags["has_g1"], has_b1=flags["has_b1"])
    nc = _CACHE[key]
    in_maps = [{**shared, **pc} for pc in per_core]
    res = run_bass_kernel_spmd(nc, in_maps, core_ids=list(range(NCORES)))
    outs = [r["out"] for r in res.results]
    full = np.concatenate(outs, axis=0).astype(np.float32)
    return full + shared["headb"].astype(np.float32)



# revision 41
# speedup vs baseline: 1.2374x; 1.0138x over previous
"""AutoregressiveMlpMixer forward on 8 Trainium2 NeuronCores (Bass/Tile).

Strategy
- Pure data parallelism: 64 batch items -> 8 per core, weights replicated.
- The reverse cumsum over tokens is folded into tok_w1 on the host
  (suffix-sum then matmul == matmul with prefix-cumsum'd weights).
- LN2 / final-LN affine params are folded into the following matmul weights
  on the host. tok_b2 is dropped exactly (it is constant along the LN2
  normalization axis, so LN2 cancels it).
- Inter-block state X is kept TRANSPOSED ([channel, token] tiles): the
  channel-MLP second matmul then accumulates its 24 k-tiles into 6
  persistent PSUM banks while E/F stream weights fused per m-tile, so the
  gelu intermediate never materializes. LN1 re-transposes X on the PE.
- Matmuls run in float32r (~13 mantissa bits, full PE rate at >=256 moving
  rows). fp8/bf16 matmuls are NOT usable: this no-residual network
  amplifies operand quantization noise ~25x (all-bf16 -> 1.7e-2 final rel
  err, fp8 channel-MLP -> 2e-1). Storage-only bf16 (X / xn / zn tiles, PE
  transposes at 1.0 cyc/row) costs ~7e-3 total and stays within budget.
- Per-engine instruction streams are in-order, so cross-engine latency
  chains are hidden by a 4-phase software pipeline emitted per step s:
  DZ(s) (LN2 transposes, inputs an iter old), A(s+2) (LN1 stats + batched
  DVE-Newton rsqrt + apply), BC(s+1) (token-MLP + LN2 stats/apply), EF(s)
  (channel-MLP, E,E,F,F per weight pair so each gelu hides under the other
  matmul). The DVE Newton rsqrt (batched over 4 LN sites) keeps
  Abs_reciprocal_sqrt off the ACT engine: no 1283ns act-table reloads.
- Channel-MLP weights stream once per G=2 items as paired-m-tile DMAs.
"""

import sys

sys.path.insert(0, "/opt/trn_rl_repo")

import numpy as np

import concourse.bass as bass
import concourse.tile as tile
from concourse import bacc, masks, mybir

f32 = mybir.dt.float32
f32r = mybir.dt.float32r
AF = mybir.ActivationFunctionType
ALU = mybir.AluOpType

# Model dims (hardcoded per problem spec)
B, CIN, H, W = 64, 2, 32, 32
N = 256          # tokens
C = 768          # hidden dim
TOK = 512        # tokens_mlp_dim
CH = 3072        # channels_mlp_dim
L = 8            # blocks
K = 2048         # classes
EPS = 1e-5

NCORES = 8
IPC = B // NCORES    # items per core = 8
NT = N // 128        # 2 token tiles per item
CT = C // 128        # 6 channel tiles
MT = CH // 128       # 24 channel-mlp tiles
TT = TOK // 128      # 4 token-mlp tiles
CC = (512, 256)      # channel free-dim chunks for 768
CCO = (0, 512)
G = 2                # items per channel-MLP weight pass


def _ln_finish(nc, pool, st, magic_t, mode="dve"):
    """bn_aggr + rsqrt. st: [128, s, 6] bn_stats. Returns (mu, rstd) APs."""
    return _ln_finish_batch(nc, pool, [st], magic_t, mode)[0]


def _ln_finish_batch(nc, pool, sts, magic_t, mode="dve"):
    """Batched bn_aggr + Newton rsqrt for n<=4 LN sites on the DVE.

    sts: list of [128, 3, 6] bn_stats tiles. Returns [(mu, rstd)] col APs.
    Batching amortizes the per-op overhead of the 9-op Newton chain."""
    i32 = mybir.dt.int32
    n = len(sts)
    # [128, 2, n]: row 0 = means, row 1 = vars -> var row is CONTIGUOUS so
    # the bitcast in the Newton iteration below is legal.
    mv = pool.tile([128, 2, n], f32, tag=f"ln_mv{n}", bufs=4, name="mv")
    for s, st in enumerate(sts):
        nc.vector.bn_aggr(out=mv[:, :, s], in_=st)
    v = mv[:, 1, :]
    if mode == "act":
        nc.scalar.activation(out=v, in_=v, func=AF.Abs_reciprocal_sqrt,
                             bias=magic_t[1], scale=1.0)
        return [(mv[:, 0, s:s + 1], v[:, s:s + 1]) for s in range(n)]
    eng = nc.gpsimd if mode == "pool" else nc.vector
    eng.tensor_scalar_add(v, v, float(EPS))
    iv = pool.tile([128, n], i32, tag=f"rs_i{n}", bufs=4, name="iv")
    eng.tensor_scalar(iv, v.bitcast(i32), 1, None,
                      ALU.logical_shift_right)
    eng.tensor_tensor(iv, magic_t[0][:, :n], iv, ALU.subtract)
    y = iv.bitcast(f32)
    t = pool.tile([128, n], f32, tag=f"rs_t{n}", bufs=4, name="t")
    for _ in range(3):
        eng.tensor_mul(t, y, y)
        eng.tensor_mul(t, t, v)
        eng.tensor_scalar(t, t, -0.5, 1.5, ALU.mult, ALU.add)
        eng.tensor_mul(y, y, t)
    return [(mv[:, 0, s:s + 1], y[:, s:s + 1]) for s in range(n)]


def _ln_stats(nc, pool, x, magic_t, mode="dve"):
    """mean/rstd of x[128, C] over the free dim. Returns (mu, rstd) col APs."""
    st = pool.tile([128, 3, 6], f32, tag="ln_st", bufs=12, name="st")
    xg = x.rearrange("p (s q) -> p s q", s=3)
    for s in range(3):
        nc.vector.bn_stats(out=st[:, s, :], in_=xg[:, s, :])
    return _ln_finish(nc, pool, st, magic_t, mode)


def build(items=IPC, blocks=L, has_g1=False, has_b1=False, kchunk=24,
          rsqrt="dve", pipelined=True):
    """Build the SPMD program for one core processing `items` batch items."""
    nc = bacc.Bacc("TRN2", target_bir_lowering=False, debug=False)

    # ---- DRAM tensors (names = in_map keys) ----
    pt = nc.dram_tensor("pt", [9, items * N], f32r, kind="ExternalInput")
    wq = nc.dram_tensor("wq", [9, C], f32r, kind="ExternalInput")
    bl = max(blocks, 1)
    tokw1c = nc.dram_tensor("tokw1c", [bl, NT, 128, TOK], f32r, kind="ExternalInput")
    tokw2 = nc.dram_tensor("tokw2", [bl, TT, 128, N], f32r, kind="ExternalInput")
    tokb1 = nc.dram_tensor("tokb1", [bl, 128, TT], f32, kind="ExternalInput")
    w1g = nc.dram_tensor("w1g", [bl, MT // 2, 128, 2, CT, 128], f32r,
                         kind="ExternalInput")
    vb1 = nc.dram_tensor("vb1", [bl, 128, MT], f32, kind="ExternalInput")
    chw2 = nc.dram_tensor("chw2", [bl, MT // 2, 128, 2, C], f32r,
                          kind="ExternalInput")
    chb2c = nc.dram_tensor("chb2c", [bl, 128, CT], f32, kind="ExternalInput")
    headwg = nc.dram_tensor("headwg", [CT, 128, K], f32r, kind="ExternalInput")
    ln1g = nc.dram_tensor("ln1g", [bl, C], f32, kind="ExternalInput")
    ln1b = nc.dram_tensor("ln1b", [bl, C], f32, kind="ExternalInput")
    out = nc.dram_tensor("out", [items, K], f32, kind="ExternalOutput")

    n_groups = (items + G - 1) // G

    with tile.TileContext(nc) as tc:
        with tc.tile_pool(name="const", bufs=1) as const, \
             tc.tile_pool(name="xstate", bufs=1) as xstate:
            magic_i = const.tile([128, 4], mybir.dt.int32, name="magic_i")
            nc.vector.memset(magic_i, 0x5F3759DF)
            eps_col = const.tile([128, 1], f32, name="eps_col")
            nc.vector.memset(eps_col, EPS)
            magic_t = (magic_i, eps_col)
            ident = const.tile([128, 128], f32, name="ident")
            masks.make_identity(nc, ident)
            identr = const.tile([128, 128], f32r, name="identr")
            nc.vector.tensor_copy(identr, ident)
            bf16 = mybir.dt.bfloat16
            identb = const.tile([128, 128], bf16, name="identb")
            nc.vector.tensor_copy(identb, ident)

            # persistent state, TRANSPOSED: X[item][ct] = [128(c), N(tokens)]
            # bf16: PE transposes run at 1.0 cyc/row and SBUF halves (the
            # ~0.2% storage noise costs ~6e-3 final rel err, within budget)
            X = [[xstate.tile([128, N], bf16, tag=f"x_{i}_{ct}",
                              name=f"x_{i}_{ct}")
                  for ct in range(CT)] for i in range(items)]

            # ---------------- mixer blocks (stem + head share the scope so
            # no pool-release barrier serializes the phase boundaries) ------
            with tc.tile_pool(name="tokw", bufs=2) as tokwp, \
                 tc.tile_pool(name="lnp", bufs=4) as lnp, \
                 tc.tile_pool(name="acts", bufs=1) as acts, \
                 tc.tile_pool(name="wstream", bufs=2) as wstream, \
                 tc.tile_pool(name="ps_mm", bufs=8, space="PSUM") as ps_mm:

                # ---- stem (writes X transposed) ----
                ptt = acts.tile([9, items * N], f32r, tag="ptt", name="ptt")
                nc.sync.dma_start(out=ptt, in_=pt[:, :])
                wqt = acts.tile([9, C], f32r, tag="wqt", name="wqt")
                nc.sync.dma_start(out=wqt, in_=wq[:, :])
                # prefetch the first channel-MLP weight pairs so block 0's
                # E matmuls don't wait on DMAs queued behind the stem
                wpf = {}
                if blocks > 0:
                    for pp in (0, 1):
                        w1g_t = wstream.tile([128, 2, CT, 128], f32r,
                                             tag="w1g", name="w1g_t")
                        nc.sync.dma_start(out=w1g_t, in_=w1g[0, pp])
                        w2c_t = wstream.tile([128, 2, C], f32r,
                                             tag="w2c", name="w2c_t")
                        nc.sync.dma_start(out=w2c_t, in_=chw2[0, pp])
                        wpf[(0, pp)] = (w1g_t, w2c_t)
                nw_all = items * N
                nchunks = [(o, min(512, nw_all - o)) for o in range(0, nw_all, 512)]
                for ct in range(CT):
                    for (no, nn) in nchunks:
                        pss = ps_mm.tile([128, 512], f32, tag="mm", name="pss")
                        nc.tensor.matmul(pss[:, :nn],
                                         wqt[:, ct * 128:(ct + 1) * 128],
                                         ptt[:, no:no + nn],
                                         start=True, stop=True)
                        for j in range(0, nn, N):
                            i = (no + j) // N
                            nc.scalar.activation(out=X[i][ct],
                                                 in_=pss[:, j:j + N],
                                                 func=AF.Copy)

                blk_w = {}

                def emit_tok_weights(l):
                    w = {}
                    w1c_t = tokwp.tile([128, NT, TOK], f32r, tag="w1c",
                                       name="w1c")
                    nc.sync.dma_start(out=w1c_t,
                                      in_=tokw1c[l].rearrange("k p t -> p k t"))
                    w2_t = tokwp.tile([128, TT, N], f32r, tag="w2", name="w2")
                    nc.sync.dma_start(out=w2_t,
                                      in_=tokw2[l].rearrange("k p n -> p k n"))
                    b1_t = tokwp.tile([128, TT], f32, tag="b1", name="b1")
                    nc.sync.dma_start(out=b1_t, in_=tokb1[l])
                    vb1_t = tokwp.tile([128, MT], f32, tag="vb1", name="vb1")
                    nc.sync.dma_start(out=vb1_t, in_=vb1[l])
                    chb2_t = tokwp.tile([128, CT], f32, tag="chb2", name="chb2")
                    nc.sync.dma_start(out=chb2_t, in_=chb2c[l])
                    w.update(w1c=w1c_t, w2=w2_t, b1=b1_t, vb1=vb1_t,
                             chb2=chb2_t)
                    if has_g1:
                        g1_t = tokwp.tile([128, C], f32, tag="g1", name="g1")
                        nc.sync.dma_start(
                            out=g1_t,
                            in_=ln1g.ap()[l:l + 1, :].partition_broadcast(128))
                        w["g1"] = g1_t
                    if has_b1:
                        b1v_t = tokwp.tile([128, C], f32, tag="b1v", name="b1v")
                        nc.sync.dma_start(
                            out=b1v_t,
                            in_=ln1b.ap()[l:l + 1, :].partition_broadcast(128))
                        w["b1v"] = b1v_t
                    return w

                def emit_A(l, g):
                    """LN1 for group g: transposes + stats + batched Newton
                    + apply -> Y tiles. Emitted 2 steps ahead of its EF so
                    the DVE Newton chain never gates the PE stream."""
                    if l not in blk_w:
                        blk_w[l] = emit_tok_weights(l)
                    g1_t = blk_w[l].get("g1")
                    b1v_t = blk_w[l].get("b1v")
                    gitems = list(range(g * G, min((g + 1) * G, items)))
                    pre = []
                    sts = []
                    for i2, i in enumerate(gitems):
                        xn = [lnp.tile([128, C], bf16, tag="xn", bufs=4,
                                       name="xn") for _ in range(NT)]
                        for t in range(NT):
                            st = lnp.tile([128, 3, 6], f32, tag="ln_st",
                                          bufs=12, name="st")
                            ptr = ps_mm.tile([128, C], bf16,
                                             tag="mm", name="ptrA")
                            for cc in range(CT):
                                nc.tensor.transpose(
                                    ptr[:, cc * 128:(cc + 1) * 128],
                                    X[i][cc][:, t * 128:(t + 1) * 128],
                                    identb)
                            nc.scalar.activation(
                                out=xn[t], in_=ptr, func=AF.Copy)
                            pgg = ptr.rearrange("p (s q) -> p s q", q=256)
                            for s in range(3):
                                nc.vector.bn_stats(
                                    out=st[:, s, :], in_=pgg[:, s, :])
                            sts.append(st)
                        pre.append(xn)
                    musall = _ln_finish_batch(nc, lnp, sts, magic_t, rsqrt)
                    Ys = []
                    for i2, i in enumerate(gitems):
                        xn = pre[i2]
                        Y = []
                        for t in range(NT):
                            mu, rstd = musall[i2 * NT + t]
                            yt = lnp.tile([128, C], f32r, tag="y", bufs=8,
                                          name="yt")
                            for cw, co in zip(CC, CCO):
                                nc.vector.tensor_scalar(
                                    out=yt[:, co:co + cw],
                                    in0=xn[t][:, co:co + cw],
                                    scalar1=mu, scalar2=rstd,
                                    op0=ALU.subtract, op1=ALU.mult)
                            if has_g1:
                                nc.vector.tensor_mul(yt, yt, g1_t)
                            if has_b1:
                                nc.vector.tensor_add(yt, yt, b1v_t)
                            Y.append(yt)
                        Ys.append(Y)
                    return Ys

                def emit_BC(l, g, Ys):
                    """token-mix + LN2 stats for group g -> (y2 tiles, rstds).
                    The LN2 apply/transpose (emit_DZ) is emitted an iteration
                    later so its Newton chain never gates the PE stream."""
                    w1c_t, w2_t, b1_t = (blk_w[l][k] for k in ("w1c", "w2", "b1"))
                    gitems = list(range(g * G, min((g + 1) * G, items)))
                    out = []
                    y1s = []
                    # ---- B for ALL items first: the last B-gelu's latency
                    # then hides under the other item's C matmuls ----
                    for i2, i in enumerate(gitems):
                        Y = Ys[i2]
                        y1 = []
                        for mt in range(TT):
                            yg = lnp.tile([128, C], f32r, tag="y1g", bufs=8,
                                          name="yg")
                            for ci, (cw, co) in enumerate(zip(CC, CCO)):
                                pb = ps_mm.tile([128, 512], f32, tag="mm",
                                                name="pb")
                                for k in range(NT):
                                    nc.tensor.matmul(
                                        pb[:, :cw],
                                        w1c_t[:, k, mt * 128:(mt + 1) * 128],
                                        Y[k][:, co:co + cw],
                                        start=(k == 0), stop=(k == NT - 1))
                                nc.scalar.activation(
                                    out=yg[:, co:co + cw], in_=pb[:, :cw],
                                    func=AF.Gelu, bias=b1_t[:, mt:mt + 1],
                                    scale=1.0)
                            y1.append(yg)
                        y1s.append(y1)
                    for i2, i in enumerate(gitems):
                        y1 = y1s[i2]
                        # ---- C: y2 = w2^T @ y1, stats from PSUM ----
                        cpost = []
                        csts = []
                        for t in range(NT):
                            y2t = lnp.tile([128, C], f32, tag="y2", bufs=4,
                                           name="y2t")
                            st = lnp.tile([128, 3, 6], f32, tag="ln_st",
                                          bufs=12, name="st")
                            for ci, (cw, co) in enumerate(zip(CC, CCO)):
                                pc = ps_mm.tile([128, 512], f32, tag="mm",
                                                name="pc")
                                for k in range(TT):
                                    nc.tensor.matmul(
                                        pc[:, :cw],
                                        w2_t[:, k, t * 128:(t + 1) * 128],
                                        y1[k][:, co:co + cw],
                                        start=(k == 0), stop=(k == TT - 1))
                                nc.scalar.activation(out=y2t[:, co:co + cw],
                                                     in_=pc[:, :cw],
                                                     func=AF.Copy)
                                # LN2 stats straight from PSUM
                                pg = pc[:, :cw].rearrange(
                                    "p (s q) -> p s q", q=256)
                                for s in range(cw // 256):
                                    nc.vector.bn_stats(
                                        out=st[:, 2 * ci + s, :],
                                        in_=pg[:, s, :])
                            cpost.append(y2t)
                            csts.append(st)
                        cmus = _ln_finish_batch(nc, lnp, csts, magic_t, rsqrt)
                        # LN2 apply now (its Newton latency hides under the
                        # concurrent EF); the transposes wait for emit_DZ.
                        zns = []
                        for t in range(NT):
                            mu, rstd = cmus[t]
                            zn = lnp.tile([128, C], bf16, tag="z", bufs=8,
                                          name="zn")
                            for cw, co in zip(CC, CCO):
                                nc.vector.tensor_scalar(
                                    out=zn[:, co:co + cw],
                                    in0=cpost[t][:, co:co + cw],
                                    scalar1=mu, scalar2=rstd,
                                    op0=ALU.subtract, op1=ALU.mult)
                            zns.append(zn)
                        out.append(zns)
                    return out

                def emit_DZ(l, g, bc):
                    """Transpose LN2 output into Zt. Emitted FIRST in its
                    iteration, one after emit_BC: the zn tiles are ready, so
                    the PE transposes and DVE copies fire immediately."""
                    Zt = acts.tile([128, CT, G * N], f32r, tag="zt",
                                   bufs=2, name="zt")
                    for i2, zns in enumerate(bc):
                        for t in range(NT):
                            zn = zns[t]
                            ptr = ps_mm.tile([128, C], bf16,
                                             tag="mm", name="ptrT")
                            for cc in range(CT):
                                nc.tensor.transpose(
                                    ptr[:, cc * 128:(cc + 1) * 128],
                                    zn[:, cc * 128:(cc + 1) * 128],
                                    identb)
                            nc.vector.tensor_copy(
                                Zt[:, :,
                                   i2 * N + t * 128:i2 * N + (t + 1) * 128],
                                ptr.rearrange("p (c q) -> p c q", q=128))
                    return Zt

                def emit_EF(l, g, Zt, kchunk=kchunk):
                    """fused channel-MLP over m-tiles for group g of block l.

                    F accumulates in PSUM per k-chunk, then folds into the
                    SBUF state X (copy w/ bias on chunk 0, add afterwards) so
                    PSUM banks are only held transiently.
                    """
                    vb1_t = blk_w[l]["vb1"]
                    chb2_t = blk_w[l]["chb2"]
                    gitems = list(range(g * G, min((g + 1) * G, items)))
                    nw = len(gitems) * N

                    for k0 in range(0, MT, kchunk):
                        psF = [ps_mm.tile([128, G * N], f32, tag="mm",
                                          name=f"pf_{ct}") for ct in range(CT)]
                        for pp in range(k0 // 2, (k0 + kchunk) // 2):
                            if (l, pp) in wpf:
                                w1g_t, w2c_t = wpf.pop((l, pp))
                            else:
                                w1g_t = wstream.tile([128, 2, CT, 128], f32r,
                                                     tag="w1g", name="w1g_t")
                                nc.sync.dma_start(out=w1g_t, in_=w1g[l, pp])
                                w2c_t = wstream.tile([128, 2, C], f32r,
                                                     tag="w2c", name="w2c_t")
                                nc.sync.dma_start(out=w2c_t, in_=chw2[l, pp])
                            # E,E then F,F per pair: each gelu's latency
                            # hides under the other matmul of the pair.
                            hgs = []
                            for j in (0, 1):
                                mt = 2 * pp + j
                                pe = ps_mm.tile([128, 512], f32, tag="mm",
                                                name="pe")
                                for kc in range(CT):
                                    nc.tensor.matmul(pe[:, :nw],
                                                     w1g_t[:, j, kc, :],
                                                     Zt[:, kc, :nw],
                                                     start=(kc == 0),
                                                     stop=(kc == CT - 1))
                                hg_cur = acts.tile([128, G * N], f32r,
                                                   tag="hg", bufs=3,
                                                   name="hg")
                                nc.scalar.activation(out=hg_cur[:, :nw],
                                                     in_=pe[:, :nw],
                                                     func=AF.Gelu,
                                                     bias=vb1_t[:, mt:mt + 1],
                                                     scale=1.0)
                                hgs.append(hg_cur)
                            for j in (0, 1):
                                mt = 2 * pp + j
                                for ct in range(CT):
                                    nc.tensor.matmul(
                                        psF[ct][:, :nw],
                                        w2c_t[:, j, ct * 128:(ct + 1) * 128],
                                        hgs[j][:, :nw],
                                        start=(mt == k0),
                                        stop=(mt == k0 + kchunk - 1))
                        for ct in range(CT):
                            for i2, i in enumerate(gitems):
                                src = psF[ct][:, i2 * N:(i2 + 1) * N]
                                if k0 == 0:
                                    nc.scalar.activation(
                                        out=X[i][ct], in_=src,
                                        func=AF.Identity,
                                        bias=chb2_t[:, ct:ct + 1], scale=1.0)
                                else:
                                    nc.vector.tensor_add(X[i][ct], X[i][ct],
                                                         src)

                # ---- head helpers (emitted per group after its last EF so
                # the final-LN work overlaps the remaining groups' EF) ----
                invn_f = acts.tile([128, 2], f32, tag="invnf", name="invn_f")
                nc.vector.memset(invn_f, 1.0 / N)
                invn_col = acts.tile([128, 2], f32r, tag="invn", name="invn")
                nc.vector.tensor_copy(invn_col, invn_f)
                xmall = acts.tile([128, CT, items], f32r, tag="xmall",
                                  name="xmall")

                def emit_head_pre(g):
                    """Final-LN transposes + stats + batched Newton for a
                    group; the apply/mean (emit_head_post) follows one EF
                    later so the rstd chain never gates the PE stream."""
                    gitems = list(range(g * G, min((g + 1) * G, items)))
                    xfs, sts = [], []
                    for i in gitems:
                        xf = [lnp.tile([128, C], bf16, tag="xf", bufs=4,
                                       name="xf") for _ in range(NT)]
                        for t in range(NT):
                            ptr = ps_mm.tile([128, C], bf16, tag="mm",
                                             name="ptrH")
                            for ct in range(CT):
                                nc.tensor.transpose(
                                    ptr[:, ct * 128:(ct + 1) * 128],
                                    X[i][ct][:, t * 128:(t + 1) * 128],
                                    identb)
                            nc.vector.tensor_copy(xf[t], ptr)
                        for t in range(NT):
                            st = lnp.tile([128, 3, 6], f32, tag="ln_st",
                                          bufs=12, name="st")
                            xg = xf[t].rearrange("p (s q) -> p s q", s=3)
                            for s in range(3):
                                nc.vector.bn_stats(out=st[:, s, :],
                                                   in_=xg[:, s, :])
                            sts.append(st)
                        xfs.append(xf)
                    mus = _ln_finish_batch(nc, lnp, sts, magic_t, rsqrt)
                    return (gitems, xfs, mus)

                def emit_head_post(hp):
                    gitems, xfs, mus = hp
                    for i2, i in enumerate(gitems):
                        xh = []
                        for t in range(NT):
                            mu, rstd = mus[i2 * NT + t]
                            xht = lnp.tile([128, C], f32r, tag="xh", bufs=2,
                                           name="xht")
                            nc.vector.tensor_scalar(
                                out=xht, in0=xfs[i2][t], scalar1=mu,
                                scalar2=rstd,
                                op0=ALU.subtract, op1=ALU.mult)
                            xh.append(xht)
                        for ct in range(CT):
                            pxm = ps_mm.tile([128, 2], f32, tag="mm",
                                             name="pxm")
                            for t in range(NT):
                                nc.tensor.matmul(
                                    pxm, xh[t][:, ct * 128:(ct + 1) * 128],
                                    invn_col,
                                    start=(t == 0), stop=(t == NT - 1))
                            nc.scalar.activation(out=xmall[:, ct, i:i + 1],
                                                 in_=pxm[:, 0:1], func=AF.Copy)

                def emit_head_final():
                    outsb = acts.tile([items, K], f32, tag="ptt",
                                      name="outsb")
                    for jc in range(K // 512):
                        ph = ps_mm.tile([items, 512], f32, tag="mm", name="ph")
                        for ct in range(CT):
                            hw_t = acts.tile([128, 512], f32r, tag="hw",
                                             bufs=4, name="hw_t")
                            nc.sync.dma_start(
                                out=hw_t,
                                in_=headwg[ct, :, jc * 512:(jc + 1) * 512])
                            nc.tensor.matmul(ph, xmall[:, ct, :items], hw_t,
                                             start=(ct == 0),
                                             stop=(ct == CT - 1))
                        nc.scalar.activation(
                            out=outsb[:, jc * 512:(jc + 1) * 512],
                            in_=ph, func=AF.Copy)
                    nc.sync.dma_start(out=out[:, :], in_=outsb)

                # software-pipelined emission: per-engine instruction streams
                # are in-order, so A (whose LN Newton chain gates B) is
                # emitted TWO steps ahead and BCD one step ahead of EF.
                seq = [(l, g) for l in range(blocks) for g in range(n_groups)]
                ys, bcs, hps = {}, {}, []
                # A(s+la) reads X written by EF(s+la-n_groups), so the A
                # lookahead must stay below n_groups (program order = dep
                # order for the tile framework). Per iteration the emission
                # order is A(s+2), BC(s+1), DZ(s), EF(s): every cross-engine
                # latency chain (LN Newton on DVE) resolves a full EF phase
                # before the PE stream needs its result.
                la = min(2, n_groups - 1) if pipelined else 0
                lb = min(1, la)
                if la > 0:
                    for k in range(min(la, len(seq))):
                        ys[seq[k]] = emit_A(*seq[k])
                    for k in range(min(lb, len(seq))):
                        bcs[seq[k]] = emit_BC(*seq[k], ys.pop(seq[k]))
                    for idx, key in enumerate(seq):
                        zt = emit_DZ(*key, bcs[key])
                        if idx + la < len(seq):
                            nkey = seq[idx + la]
                            ys[nkey] = emit_A(*nkey)
                        if idx + lb < len(seq):
                            nkey = seq[idx + lb]
                            if nkey not in bcs:
                                bcs[nkey] = emit_BC(*nkey, ys.pop(nkey))
                        bcs.pop(key)
                        emit_EF(*key, zt)
                        if key[0] == blocks - 1:
                            if hps:
                                emit_head_post(hps.pop(0))
                            hps.append(emit_head_pre(key[1]))
                else:
                    for key in seq:
                        emit_EF(*key, emit_DZ(*key, emit_BC(*key,
                                                            emit_A(*key))))
                        if key[0] == blocks - 1:
                            if hps:
                                emit_head_post(hps.pop(0))
                            hps.append(emit_head_pre(key[1]))
                if not seq:
                    for g in range(n_groups):
                        hps.append(emit_head_pre(g))
                while hps:
                    emit_head_post(hps.pop(0))
                emit_head_final()

    nc.compile()
    return nc


# ---------------------------------------------------------------------------
# host-side preprocessing
# ---------------------------------------------------------------------------

def prep_inputs(inputs, stem_w, stem_b, ln1_g, ln1_b, tok_w1, tok_b1, tok_w2,
                tok_b2, ln2_g, ln2_b, ch_w1, ch_b1, ch_w2, ch_b2, lnf_g, lnf_b,
                head_w, head_b, items=IPC, blocks=L):
    """Returns (shared_map, per_core_list, flags)."""
    f = np.float32
    inputs = np.asarray(inputs, f)
    # patches: (B, CIN, 16, 2, 16, 2) -> (B, n=256, q=8); +ones row -> (B,9,256)
    x = inputs.reshape(B, CIN, H // 2, 2, W // 2, 2).transpose(0, 2, 4, 1, 3, 5)
    x = x.reshape(B, N, CIN * 4)
    ptA = np.concatenate([x.transpose(0, 2, 1),
                          np.ones((B, 1, N), f)], axis=1)  # (B, 9, 256)

    wq = np.concatenate([np.asarray(stem_w, f).reshape(C, 8).T,
                         np.asarray(stem_b, f)[None, :]], axis=0)  # (9, C)

    blocks = max(blocks, 1)
    w1cum = np.cumsum(np.asarray(tok_w1, f), axis=1)[:blocks]        # (L, N, TOK)
    tokw1c = np.ascontiguousarray(w1cum.reshape(blocks, NT, 128, TOK))
    tokw2 = np.ascontiguousarray(np.asarray(tok_w2, f)[:blocks]
                                 .reshape(blocks, TT, 128, N))
    tokb1 = np.ascontiguousarray(np.asarray(tok_b1, f)[:blocks]
                                 .reshape(blocks, TT, 128).transpose(0, 2, 1))

    g2 = np.asarray(ln2_g, f)[:blocks]
    b2 = np.asarray(ln2_b, f)[:blocks]
    cw1 = np.asarray(ch_w1, f)[:blocks]
    w1g_full = g2[:, :, None] * cw1                                   # (L, C, CH)
    w1g = (w1g_full.reshape(blocks, CT, 128, MT, 128)
           .transpose(0, 3, 2, 1, 4)                 # (L, MT, 128, CT, 128)
           .reshape(blocks, MT // 2, 2, 128, CT, 128)
           .transpose(0, 1, 3, 2, 4, 5))             # (L, 12, 128, 2, CT, 128)
    w1g = np.ascontiguousarray(w1g)
    v = np.einsum("lc,lcm->lm", b2, cw1) + np.asarray(ch_b1, f)[:blocks]
    vb1 = np.ascontiguousarray(v.reshape(blocks, MT, 128).transpose(0, 2, 1))
    chw2 = (np.asarray(ch_w2, f)[:blocks]
            .reshape(blocks, MT // 2, 2, 128, C)
            .transpose(0, 1, 3, 2, 4))               # (L, 12, 128, 2, C)
    chw2 = np.ascontiguousarray(chw2)
    chb2c = np.ascontiguousarray(np.asarray(ch_b2, f)[:blocks]
                                 .reshape(blocks, CT, 128).transpose(0, 2, 1))

    gf = np.asarray(lnf_g, f)
    bf = np.asarray(lnf_b, f)
    hw = np.asarray(head_w, f)
    headwg = np.ascontiguousarray((gf[:, None] * hw).reshape(CT, 128, K))
    headb = (bf @ hw + np.asarray(head_b, f)).reshape(1, K).astype(f)

    ln1g = np.ascontiguousarray(np.asarray(ln1_g, f)[:blocks])
    ln1b = np.ascontiguousarray(np.asarray(ln1_b, f)[:blocks])
    has_g1 = not np.all(ln1g == 1.0)
    has_b1 = not np.all(ln1b == 0.0)

    shared = dict(wq=wq, tokw1c=tokw1c, tokw2=tokw2, tokb1=tokb1, w1g=w1g,
                  vb1=vb1, chw2=chw2, chb2c=chb2c, headwg=headwg, headb=headb,
                  ln1g=ln1g, ln1b=ln1b)
    shared = {k: np.ascontiguousarray(v, f) for k, v in shared.items()}

    per_core = []
    for c in range(NCORES):
        sel = ptA[c * IPC:(c + 1) * IPC][:items]  # (items, 9, 256)
        ptc = np.ascontiguousarray(sel.transpose(1, 0, 2).reshape(9, items * N))
        per_core.append(dict(pt=ptc))
    return shared, per_core, dict(has_g1=has_g1, has_b1=has_b1)


_CACHE = {}


def kernel(**inputs):
    from concourse.bass_utils import run_bass_kernel_spmd
    shared, per_core, flags = prep_inputs(**inputs)
    key = (flags["has_g1"], flags["has_b1"])
    if key not in _CACHE:
        _CACHE[key] = build(has_g1=flags["has_g1"], has_b1=flags["has_b1"])
    nc = _CACHE[key]
    in_maps = [{**shared, **pc} for pc in per_core]
    res = run_bass_kernel_spmd(nc, in_maps, core_ids=list(range(NCORES)))
    outs = [r["out"] for r in res.results]
    full = np.concatenate(outs, axis=0).astype(np.float32)
    return full + shared["headb"].astype(np.float32)



# revision 54
# speedup vs baseline: 1.2383x; 1.0007x over previous
"""AutoregressiveMlpMixer forward on 8 Trainium2 NeuronCores (Bass/Tile).

Strategy
- Pure data parallelism: 64 batch items -> 8 per core, weights replicated.
- The reverse cumsum over tokens is folded into tok_w1 on the host
  (suffix-sum then matmul == matmul with prefix-cumsum'd weights).
- LN2 / final-LN affine params are folded into the following matmul weights
  on the host. tok_b2 is dropped exactly (it is constant along the LN2
  normalization axis, so LN2 cancels it).
- Inter-block state X is kept TRANSPOSED ([channel, token] tiles): the
  channel-MLP second matmul then accumulates its 24 k-tiles into 6
  persistent PSUM banks while E/F stream weights fused per m-tile, so the
  gelu intermediate never materializes. LN1 re-transposes X on the PE.
- Matmuls run in float32r (~13 mantissa bits, full PE rate at >=256 moving
  rows). fp8/bf16 matmuls are NOT usable: this no-residual network
  amplifies operand quantization noise ~25x (all-bf16 -> 1.7e-2 final rel
  err, fp8 channel-MLP -> 2e-1). Storage-only bf16 (X / xn / zn tiles, PE
  transposes at 1.0 cyc/row) costs ~7e-3 total and stays within budget.
- Per-engine instruction streams are in-order, so cross-engine latency
  chains are hidden by a 4-phase software pipeline emitted per step s:
  DZ(s) (LN2 transposes, inputs an iter old), A(s+2) (LN1 stats + batched
  DVE-Newton rsqrt + apply), BC(s+1) (token-MLP + LN2 stats/apply), EF(s)
  (channel-MLP, E,E,F,F per weight pair so each gelu hides under the other
  matmul). The DVE Newton rsqrt (batched over 4 LN sites) keeps
  Abs_reciprocal_sqrt off the ACT engine: no 1283ns act-table reloads.
- Channel-MLP weights stream once per G=2 items as paired-m-tile DMAs.
"""

import sys

sys.path.insert(0, "/opt/trn_rl_repo")

import numpy as np

import concourse.bass as bass
import concourse.tile as tile
from concourse import bacc, masks, mybir

f32 = mybir.dt.float32
f32r = mybir.dt.float32r
AF = mybir.ActivationFunctionType
ALU = mybir.AluOpType

# Model dims (hardcoded per problem spec)
B, CIN, H, W = 64, 2, 32, 32
N = 256          # tokens
C = 768          # hidden dim
TOK = 512        # tokens_mlp_dim
CH = 3072        # channels_mlp_dim
L = 8            # blocks
K = 2048         # classes
EPS = 1e-5

NCORES = 8
IPC = B // NCORES    # items per core = 8
NT = N // 128        # 2 token tiles per item
CT = C // 128        # 6 channel tiles
MT = CH // 128       # 24 channel-mlp tiles
TT = TOK // 128      # 4 token-mlp tiles
CC = (512, 256)      # channel free-dim chunks for 768
CCO = (0, 512)
G = 2                # items per channel-MLP weight pass


def _ln_finish(nc, pool, st, magic_t, mode="dve"):
    """bn_aggr + rsqrt. st: [128, s, 6] bn_stats. Returns (mu, rstd) APs."""
    return _ln_finish_batch(nc, pool, [st], magic_t, mode)[0]


def _ln_finish_batch(nc, pool, sts, magic_t, mode="dve"):
    """Batched bn_aggr + Newton rsqrt for n<=4 LN sites on the DVE.

    sts: list of [128, 3, 6] bn_stats tiles. Returns [(mu, rstd)] col APs.
    Batching amortizes the per-op overhead of the 9-op Newton chain."""
    i32 = mybir.dt.int32
    n = len(sts)
    # [128, 2, n]: row 0 = means, row 1 = vars -> var row is CONTIGUOUS so
    # the bitcast in the Newton iteration below is legal.
    mv = pool.tile([128, 2, n], f32, tag=f"ln_mv{n}", bufs=4, name="mv")
    for s, st in enumerate(sts):
        nc.vector.bn_aggr(out=mv[:, :, s], in_=st)
    v = mv[:, 1, :]
    if mode == "act":
        nc.scalar.activation(out=v, in_=v, func=AF.Abs_reciprocal_sqrt,
                             bias=magic_t[1], scale=1.0)
        return [(mv[:, 0, s:s + 1], v[:, s:s + 1]) for s in range(n)]
    eng = nc.gpsimd if mode == "pool" else nc.vector
    eng.tensor_scalar_add(v, v, float(EPS))
    iv = pool.tile([128, n], i32, tag=f"rs_i{n}", bufs=4, name="iv")
    eng.tensor_scalar(iv, v.bitcast(i32), 1, None,
                      ALU.logical_shift_right)
    eng.tensor_tensor(iv, magic_t[0][:, :n], iv, ALU.subtract)
    y = iv.bitcast(f32)
    t = pool.tile([128, n], f32, tag=f"rs_t{n}", bufs=4, name="t")
    for _ in range(3):
        eng.tensor_mul(t, y, y)
        eng.tensor_mul(t, t, v)
        eng.tensor_scalar(t, t, -0.5, 1.5, ALU.mult, ALU.add)
        eng.tensor_mul(y, y, t)
    return [(mv[:, 0, s:s + 1], y[:, s:s + 1]) for s in range(n)]


def _ln_stats(nc, pool, x, magic_t, mode="dve"):
    """mean/rstd of x[128, C] over the free dim. Returns (mu, rstd) col APs."""
    st = pool.tile([128, 3, 6], f32, tag="ln_st", bufs=12, name="st")
    xg = x.rearrange("p (s q) -> p s q", s=3)
    for s in range(3):
        nc.vector.bn_stats(out=st[:, s, :], in_=xg[:, s, :])
    return _ln_finish(nc, pool, st, magic_t, mode)


def build(items=IPC, blocks=L, has_g1=False, has_b1=False, kchunk=24,
          rsqrt="dve", pipelined=True):
    """Build the SPMD program for one core processing `items` batch items."""
    nc = bacc.Bacc("TRN2", target_bir_lowering=False, debug=False)

    # ---- DRAM tensors (names = in_map keys) ----
    pt = nc.dram_tensor("pt", [9, items * N], f32r, kind="ExternalInput")
    wq = nc.dram_tensor("wq", [9, C], f32r, kind="ExternalInput")
    bl = max(blocks, 1)
    tokw1c = nc.dram_tensor("tokw1c", [bl, NT, 128, TOK], f32r, kind="ExternalInput")
    tokw2 = nc.dram_tensor("tokw2", [bl, TT, 128, N], f32r, kind="ExternalInput")
    tokb1 = nc.dram_tensor("tokb1", [bl, 128, TT], f32, kind="ExternalInput")
    w1g = nc.dram_tensor("w1g", [bl, MT // 2, 128, 2, CT, 128], f32r,
                         kind="ExternalInput")
    vb1 = nc.dram_tensor("vb1", [bl, 128, MT], f32, kind="ExternalInput")
    chw2 = nc.dram_tensor("chw2", [bl, MT // 2, 128, 2, C], f32r,
                          kind="ExternalInput")
    chb2c = nc.dram_tensor("chb2c", [bl, 128, CT], f32, kind="ExternalInput")
    headwg = nc.dram_tensor("headwg", [CT, 128, K], mybir.dt.bfloat16,
                            kind="ExternalInput")
    ln1g = nc.dram_tensor("ln1g", [bl, C], f32, kind="ExternalInput")
    ln1b = nc.dram_tensor("ln1b", [bl, C], f32, kind="ExternalInput")
    out = nc.dram_tensor("out", [items, K], f32, kind="ExternalOutput")

    n_groups = (items + G - 1) // G

    with tile.TileContext(nc) as tc:
        with tc.tile_pool(name="const", bufs=1) as const, \
             tc.tile_pool(name="xstate", bufs=1) as xstate:
            magic_i = const.tile([128, 4], mybir.dt.int32, name="magic_i")
            nc.vector.memset(magic_i, 0x5F3759DF)
            eps_col = const.tile([128, 1], f32, name="eps_col")
            nc.vector.memset(eps_col, EPS)
            magic_t = (magic_i, eps_col)
            ident = const.tile([128, 128], f32, name="ident")
            masks.make_identity(nc, ident)
            identr = const.tile([128, 128], f32r, name="identr")
            nc.vector.tensor_copy(identr, ident)
            bf16 = mybir.dt.bfloat16
            identb = const.tile([128, 128], bf16, name="identb")
            nc.vector.tensor_copy(identb, ident)

            # persistent state, TRANSPOSED: X[item][ct] = [128(c), N(tokens)]
            # bf16: PE transposes run at 1.0 cyc/row and SBUF halves (the
            # ~0.2% storage noise costs ~6e-3 final rel err, within budget)
            X = [[xstate.tile([128, N], bf16, tag=f"x_{i}_{ct}",
                              name=f"x_{i}_{ct}")
                  for ct in range(CT)] for i in range(items)]

            # ---------------- mixer blocks (stem + head share the scope so
            # no pool-release barrier serializes the phase boundaries) ------
            with tc.tile_pool(name="tokw", bufs=2) as tokwp, \
                 tc.tile_pool(name="lnp", bufs=4) as lnp, \
                 tc.tile_pool(name="acts", bufs=1) as acts, \
                 tc.tile_pool(name="wstream", bufs=2) as wstream, \
                 tc.tile_pool(name="ps_mm", bufs=8, space="PSUM") as ps_mm:

                # ---- stem (writes X transposed) ----
                ptt = acts.tile([9, items * N], f32r, tag="ptt", name="ptt")
                nc.sync.dma_start(out=ptt, in_=pt[:, :])
                wqt = acts.tile([9, C], f32r, tag="wqt", name="wqt")
                nc.sync.dma_start(out=wqt, in_=wq[:, :])
                # prefetch the first channel-MLP weight pairs so block 0's
                # E matmuls don't wait on DMAs queued behind the stem
                wpf = {}
                if blocks > 0:
                    for pp in (0, 1):
                        w1g_t = wstream.tile([128, 2, CT, 128], f32r,
                                             tag="w1g", name="w1g_t")
                        nc.sync.dma_start(out=w1g_t, in_=w1g[0, pp])
                        w2c_t = wstream.tile([128, 2, C], f32r,
                                             tag="w2c", name="w2c_t")
                        nc.sync.dma_start(out=w2c_t, in_=chw2[0, pp])
                        wpf[(0, pp)] = (w1g_t, w2c_t)
                nw_all = items * N
                nchunks = [(o, min(512, nw_all - o)) for o in range(0, nw_all, 512)]
                for ct in range(CT):
                    for (no, nn) in nchunks:
                        pss = ps_mm.tile([128, 512], f32, tag="mm", name="pss")
                        nc.tensor.matmul(pss[:, :nn],
                                         wqt[:, ct * 128:(ct + 1) * 128],
                                         ptt[:, no:no + nn],
                                         start=True, stop=True)
                        for j in range(0, nn, N):
                            i = (no + j) // N
                            nc.scalar.activation(out=X[i][ct],
                                                 in_=pss[:, j:j + N],
                                                 func=AF.Copy)

                blk_w = {}

                def emit_tok_weights(l):
                    w = {}
                    w1c_t = tokwp.tile([128, NT, TOK], f32r, tag="w1c",
                                       name="w1c")
                    nc.sync.dma_start(out=w1c_t,
                                      in_=tokw1c[l].rearrange("k p t -> p k t"))
                    w2_t = tokwp.tile([128, TT, N], f32r, tag="w2", name="w2")
                    nc.sync.dma_start(out=w2_t,
                                      in_=tokw2[l].rearrange("k p n -> p k n"))
                    b1_t = tokwp.tile([128, TT], f32, tag="b1", name="b1")
                    nc.sync.dma_start(out=b1_t, in_=tokb1[l])
                    vb1_t = tokwp.tile([128, MT], f32, tag="vb1", name="vb1")
                    nc.sync.dma_start(out=vb1_t, in_=vb1[l])
                    chb2_t = tokwp.tile([128, CT], f32, tag="chb2", name="chb2")
                    nc.sync.dma_start(out=chb2_t, in_=chb2c[l])
                    w.update(w1c=w1c_t, w2=w2_t, b1=b1_t, vb1=vb1_t,
                             chb2=chb2_t)
                    if has_g1:
                        g1_t = tokwp.tile([128, C], f32, tag="g1", name="g1")
                        nc.sync.dma_start(
                            out=g1_t,
                            in_=ln1g.ap()[l:l + 1, :].partition_broadcast(128))
                        w["g1"] = g1_t
                    if has_b1:
                        b1v_t = tokwp.tile([128, C], f32, tag="b1v", name="b1v")
                        nc.sync.dma_start(
                            out=b1v_t,
                            in_=ln1b.ap()[l:l + 1, :].partition_broadcast(128))
                        w["b1v"] = b1v_t
                    return w

                def emit_A(l, g):
                    """LN1 for group g: transposes + stats + batched Newton
                    + apply -> Y tiles. Emitted 2 steps ahead of its EF so
                    the DVE Newton chain never gates the PE stream."""
                    if l not in blk_w:
                        blk_w[l] = emit_tok_weights(l)
                    g1_t = blk_w[l].get("g1")
                    b1v_t = blk_w[l].get("b1v")
                    gitems = list(range(g * G, min((g + 1) * G, items)))
                    pre = []
                    sts = []
                    for i2, i in enumerate(gitems):
                        xn = [lnp.tile([128, C], bf16, tag="xn", bufs=4,
                                       name="xn") for _ in range(NT)]
                        for t in range(NT):
                            st = lnp.tile([128, 3, 6], f32, tag="ln_st",
                                          bufs=12, name="st")
                            ptr = ps_mm.tile([128, C], bf16,
                                             tag="mm", name="ptrA")
                            for cc in range(CT):
                                nc.tensor.transpose(
                                    ptr[:, cc * 128:(cc + 1) * 128],
                                    X[i][cc][:, t * 128:(t + 1) * 128],
                                    identb)
                            nc.scalar.activation(
                                out=xn[t], in_=ptr, func=AF.Copy)
                            pgg = ptr.rearrange("p (s q) -> p s q", q=256)
                            for s in range(3):
                                nc.vector.bn_stats(
                                    out=st[:, s, :], in_=pgg[:, s, :])
                            sts.append(st)
                        pre.append(xn)
                    musall = _ln_finish_batch(nc, lnp, sts, magic_t, rsqrt)
                    Ys = []
                    for i2, i in enumerate(gitems):
                        xn = pre[i2]
                        Y = []
                        for t in range(NT):
                            mu, rstd = musall[i2 * NT + t]
                            yt = lnp.tile([128, C], f32r, tag="y", bufs=8,
                                          name="yt")
                            for cw, co in zip(CC, CCO):
                                nc.vector.tensor_scalar(
                                    out=yt[:, co:co + cw],
                                    in0=xn[t][:, co:co + cw],
                                    scalar1=mu, scalar2=rstd,
                                    op0=ALU.subtract, op1=ALU.mult)
                            if has_g1:
                                nc.vector.tensor_mul(yt, yt, g1_t)
                            if has_b1:
                                nc.vector.tensor_add(yt, yt, b1v_t)
                            Y.append(yt)
                        Ys.append(Y)
                    return Ys

                def emit_BC(l, g, Ys):
                    """token-mix + LN2 stats for group g -> (y2 tiles, rstds).
                    The LN2 apply/transpose (emit_DZ) is emitted an iteration
                    later so its Newton chain never gates the PE stream."""
                    w1c_t, w2_t, b1_t = (blk_w[l][k] for k in ("w1c", "w2", "b1"))
                    gitems = list(range(g * G, min((g + 1) * G, items)))
                    out = []
                    y1s = []
                    # ---- B for ALL items first: the last B-gelu's latency
                    # then hides under the other item's C matmuls ----
                    for i2, i in enumerate(gitems):
                        Y = Ys[i2]
                        y1 = []
                        for mt in range(TT):
                            yg = lnp.tile([128, C], f32r, tag="y1g", bufs=8,
                                          name="yg")
                            for ci, (cw, co) in enumerate(zip(CC, CCO)):
                                pb = ps_mm.tile([128, 512], f32, tag="mm",
                                                name="pb")
                                for k in range(NT):
                                    nc.tensor.matmul(
                                        pb[:, :cw],
                                        w1c_t[:, k, mt * 128:(mt + 1) * 128],
                                        Y[k][:, co:co + cw],
                                        start=(k == 0), stop=(k == NT - 1))
                                nc.scalar.activation(
                                    out=yg[:, co:co + cw], in_=pb[:, :cw],
                                    func=AF.Gelu, bias=b1_t[:, mt:mt + 1],
                                    scale=1.0)
                            y1.append(yg)
                        y1s.append(y1)
                    for i2, i in enumerate(gitems):
                        y1 = y1s[i2]
                        # ---- C: y2 = w2^T @ y1, stats from PSUM ----
                        cpost = []
                        csts = []
                        for t in range(NT):
                            y2t = lnp.tile([128, C], f32, tag="y2", bufs=4,
                                           name="y2t")
                            st = lnp.tile([128, 3, 6], f32, tag="ln_st",
                                          bufs=12, name="st")
                            for ci, (cw, co) in enumerate(zip(CC, CCO)):
                                pc = ps_mm.tile([128, 512], f32, tag="mm",
                                                name="pc")
                                for k in range(TT):
                                    nc.tensor.matmul(
                                        pc[:, :cw],
                                        w2_t[:, k, t * 128:(t + 1) * 128],
                                        y1[k][:, co:co + cw],
                                        start=(k == 0), stop=(k == TT - 1))
                                nc.scalar.activation(out=y2t[:, co:co + cw],
                                                     in_=pc[:, :cw],
                                                     func=AF.Copy)
                                # LN2 stats straight from PSUM
                                pg = pc[:, :cw].rearrange(
                                    "p (s q) -> p s q", q=256)
                                for s in range(cw // 256):
                                    nc.vector.bn_stats(
                                        out=st[:, 2 * ci + s, :],
                                        in_=pg[:, s, :])
                            cpost.append(y2t)
                            csts.append(st)
                        cmus = _ln_finish_batch(nc, lnp, csts, magic_t, rsqrt)
                        # LN2 apply now (its Newton latency hides under the
                        # concurrent EF); the transposes wait for emit_DZ.
                        zns = []
                        for t in range(NT):
                            mu, rstd = cmus[t]
                            zn = lnp.tile([128, C], bf16, tag="z", bufs=8,
                                          name="zn")
                            for cw, co in zip(CC, CCO):
                                nc.vector.tensor_scalar(
                                    out=zn[:, co:co + cw],
                                    in0=cpost[t][:, co:co + cw],
                                    scalar1=mu, scalar2=rstd,
                                    op0=ALU.subtract, op1=ALU.mult)
                            zns.append(zn)
                        out.append(zns)
                    return out

                def emit_DZ(l, g, bc):
                    """Transpose LN2 output into Zt. Emitted FIRST in its
                    iteration, one after emit_BC: the zn tiles are ready, so
                    the PE transposes and DVE copies fire immediately."""
                    Zt = acts.tile([128, CT, G * N], f32r, tag="zt",
                                   bufs=2, name="zt")
                    for i2, zns in enumerate(bc):
                        for t in range(NT):
                            zn = zns[t]
                            ptr = ps_mm.tile([128, C], bf16,
                                             tag="mm", name="ptrT")
                            for cc in range(CT):
                                nc.tensor.transpose(
                                    ptr[:, cc * 128:(cc + 1) * 128],
                                    zn[:, cc * 128:(cc + 1) * 128],
                                    identb)
                            nc.vector.tensor_copy(
                                Zt[:, :,
                                   i2 * N + t * 128:i2 * N + (t + 1) * 128],
                                ptr.rearrange("p (c q) -> p c q", q=128))
                    return Zt

                def emit_EF(l, g, Zt, kchunk=kchunk):
                    """fused channel-MLP over m-tiles for group g of block l.

                    F accumulates in PSUM per k-chunk, then folds into the
                    SBUF state X (copy w/ bias on chunk 0, add afterwards) so
                    PSUM banks are only held transiently.
                    """
                    vb1_t = blk_w[l]["vb1"]
                    chb2_t = blk_w[l]["chb2"]
                    gitems = list(range(g * G, min((g + 1) * G, items)))
                    nw = len(gitems) * N

                    for k0 in range(0, MT, kchunk):
                        psF = [ps_mm.tile([128, G * N], f32, tag="mm",
                                          name=f"pf_{ct}") for ct in range(CT)]
                        for pp in range(k0 // 2, (k0 + kchunk) // 2):
                            if (l, pp) in wpf:
                                w1g_t, w2c_t = wpf.pop((l, pp))
                            else:
                                w1g_t = wstream.tile([128, 2, CT, 128], f32r,
                                                     tag="w1g", name="w1g_t")
                                nc.sync.dma_start(out=w1g_t, in_=w1g[l, pp])
                                w2c_t = wstream.tile([128, 2, C], f32r,
                                                     tag="w2c", name="w2c_t")
                                nc.sync.dma_start(out=w2c_t, in_=chw2[l, pp])
                            # E,E then F,F per pair: each gelu's latency
                            # hides under the other matmul of the pair.
                            hgs = []
                            for j in (0, 1):
                                mt = 2 * pp + j
                                pe = ps_mm.tile([128, 512], f32, tag="mm",
                                                name="pe")
                                for kc in range(CT):
                                    nc.tensor.matmul(pe[:, :nw],
                                                     w1g_t[:, j, kc, :],
                                                     Zt[:, kc, :nw],
                                                     start=(kc == 0),
                                                     stop=(kc == CT - 1))
                                hg_cur = acts.tile([128, G * N], f32r,
                                                   tag="hg", bufs=3,
                                                   name="hg")
                                nc.scalar.activation(out=hg_cur[:, :nw],
                                                     in_=pe[:, :nw],
                                                     func=AF.Gelu,
                                                     bias=vb1_t[:, mt:mt + 1],
                                                     scale=1.0)
                                hgs.append(hg_cur)
                            for j in (0, 1):
                                mt = 2 * pp + j
                                for ct in range(CT):
                                    nc.tensor.matmul(
                                        psF[ct][:, :nw],
                                        w2c_t[:, j, ct * 128:(ct + 1) * 128],
                                        hgs[j][:, :nw],
                                        start=(mt == k0),
                                        stop=(mt == k0 + kchunk - 1))
                        for ct in range(CT):
                            for i2, i in enumerate(gitems):
                                src = psF[ct][:, i2 * N:(i2 + 1) * N]
                                if k0 == 0:
                                    nc.scalar.activation(
                                        out=X[i][ct], in_=src,
                                        func=AF.Identity,
                                        bias=chb2_t[:, ct:ct + 1], scale=1.0)
                                else:
                                    nc.vector.tensor_add(X[i][ct], X[i][ct],
                                                         src)

                # ---- head helpers (emitted per group after its last EF so
                # the final-LN work overlaps the remaining groups' EF) ----
                invn_f = acts.tile([128, 2], f32, tag="invnf", name="invn_f")
                nc.vector.memset(invn_f, 1.0 / N)
                invn_col = acts.tile([128, 2], f32r, tag="invn", name="invn")
                nc.vector.tensor_copy(invn_col, invn_f)
                xmall = acts.tile([128, CT, items], bf16, tag="xmall",
                                  name="xmall")

                def emit_head_pre(g):
                    """Final-LN transposes + stats + batched Newton for a
                    group; the apply/mean (emit_head_post) follows one EF
                    later so the rstd chain never gates the PE stream."""
                    gitems = list(range(g * G, min((g + 1) * G, items)))
                    xfs, sts = [], []
                    for i in gitems:
                        xf = [lnp.tile([128, C], bf16, tag="xf", bufs=4,
                                       name="xf") for _ in range(NT)]
                        for t in range(NT):
                            ptr = ps_mm.tile([128, C], bf16, tag="mm",
                                             name="ptrH")
                            for ct in range(CT):
                                nc.tensor.transpose(
                                    ptr[:, ct * 128:(ct + 1) * 128],
                                    X[i][ct][:, t * 128:(t + 1) * 128],
                                    identb)
                            nc.vector.tensor_copy(xf[t], ptr)
                        for t in range(NT):
                            st = lnp.tile([128, 3, 6], f32, tag="ln_st",
                                          bufs=12, name="st")
                            xg = xf[t].rearrange("p (s q) -> p s q", s=3)
                            for s in range(3):
                                nc.vector.bn_stats(out=st[:, s, :],
                                                   in_=xg[:, s, :])
                            sts.append(st)
                        xfs.append(xf)
                    mus = _ln_finish_batch(nc, lnp, sts, magic_t, rsqrt)
                    return (gitems, xfs, mus)

                def emit_head_post(hp):
                    gitems, xfs, mus = hp
                    for i2, i in enumerate(gitems):
                        xh = []
                        for t in range(NT):
                            mu, rstd = mus[i2 * NT + t]
                            xht = lnp.tile([128, C], f32r, tag="xh", bufs=2,
                                           name="xht")
                            nc.vector.tensor_scalar(
                                out=xht, in0=xfs[i2][t], scalar1=mu,
                                scalar2=rstd,
                                op0=ALU.subtract, op1=ALU.mult)
                            xh.append(xht)
                        for ct in range(CT):
                            pxm = ps_mm.tile([128, 2], f32, tag="mm",
                                             name="pxm")
                            for t in range(NT):
                                nc.tensor.matmul(
                                    pxm, xh[t][:, ct * 128:(ct + 1) * 128],
                                    invn_col,
                                    start=(t == 0), stop=(t == NT - 1))
                            nc.scalar.activation(out=xmall[:, ct, i:i + 1],
                                                 in_=pxm[:, 0:1], func=AF.Copy)

                def emit_head_final():
                    outsb = acts.tile([items, K], f32, tag="ptt",
                                      name="outsb")
                    for jc in range(K // 512):
                        ph = ps_mm.tile([items, 512], f32, tag="mm", name="ph")
                        for ct in range(CT):
                            hw_t = acts.tile([128, 512], bf16, tag="hw",
                                             bufs=4, name="hw_t")
                            nc.sync.dma_start(
                                out=hw_t,
                                in_=headwg[ct, :, jc * 512:(jc + 1) * 512])
                            nc.tensor.matmul(ph, xmall[:, ct, :items], hw_t,
                                             start=(ct == 0),
                                             stop=(ct == CT - 1))
                        nc.scalar.activation(
                            out=outsb[:, jc * 512:(jc + 1) * 512],
                            in_=ph, func=AF.Copy)
                    nc.sync.dma_start(out=out[:, :], in_=outsb)

                # software-pipelined emission: per-engine instruction streams
                # are in-order, so A (whose LN Newton chain gates B) is
                # emitted TWO steps ahead and BCD one step ahead of EF.
                seq = [(l, g) for l in range(blocks) for g in range(n_groups)]
                ys, bcs, hps = {}, {}, []
                # A(s+la) reads X written by EF(s+la-n_groups), so the A
                # lookahead must stay below n_groups (program order = dep
                # order for the tile framework). Per iteration the emission
                # order is A(s+2), BC(s+1), DZ(s), EF(s): every cross-engine
                # latency chain (LN Newton on DVE) resolves a full EF phase
                # before the PE stream needs its result.
                la = min(2, n_groups - 1) if pipelined else 0
                lb = min(1, la)
                if la > 0:
                    for k in range(min(la, len(seq))):
                        ys[seq[k]] = emit_A(*seq[k])
                    for k in range(min(lb, len(seq))):
                        bcs[seq[k]] = emit_BC(*seq[k], ys.pop(seq[k]))
                    for idx, key in enumerate(seq):
                        zt = emit_DZ(*key, bcs[key])
                        if idx + la < len(seq):
                            nkey = seq[idx + la]
                            ys[nkey] = emit_A(*nkey)
                        if idx + lb < len(seq):
                            nkey = seq[idx + lb]
                            if nkey not in bcs:
                                bcs[nkey] = emit_BC(*nkey, ys.pop(nkey))
                        bcs.pop(key)
                        emit_EF(*key, zt)
                        if key[0] == blocks - 1:
                            if hps:
                                emit_head_post(hps.pop(0))
                            hps.append(emit_head_pre(key[1]))
                else:
                    for key in seq:
                        emit_EF(*key, emit_DZ(*key, emit_BC(*key,
                                                            emit_A(*key))))
                        if key[0] == blocks - 1:
                            if hps:
                                emit_head_post(hps.pop(0))
                            hps.append(emit_head_pre(key[1]))
                if not seq:
                    for g in range(n_groups):
                        hps.append(emit_head_pre(g))
                while hps:
                    emit_head_post(hps.pop(0))
                emit_head_final()

    nc.compile()
    return nc


# ---------------------------------------------------------------------------
# host-side preprocessing
# ---------------------------------------------------------------------------

def prep_inputs(inputs, stem_w, stem_b, ln1_g, ln1_b, tok_w1, tok_b1, tok_w2,
                tok_b2, ln2_g, ln2_b, ch_w1, ch_b1, ch_w2, ch_b2, lnf_g, lnf_b,
                head_w, head_b, items=IPC, blocks=L):
    """Returns (shared_map, per_core_list, flags)."""
    f = np.float32
    inputs = np.asarray(inputs, f)
    # patches: (B, CIN, 16, 2, 16, 2) -> (B, n=256, q=8); +ones row -> (B,9,256)
    x = inputs.reshape(B, CIN, H // 2, 2, W // 2, 2).transpose(0, 2, 4, 1, 3, 5)
    x = x.reshape(B, N, CIN * 4)
    ptA = np.concatenate([x.transpose(0, 2, 1),
                          np.ones((B, 1, N), f)], axis=1)  # (B, 9, 256)

    wq = np.concatenate([np.asarray(stem_w, f).reshape(C, 8).T,
                         np.asarray(stem_b, f)[None, :]], axis=0)  # (9, C)

    blocks = max(blocks, 1)
    w1cum = np.cumsum(np.asarray(tok_w1, f), axis=1)[:blocks]        # (L, N, TOK)
    tokw1c = np.ascontiguousarray(w1cum.reshape(blocks, NT, 128, TOK))
    tokw2 = np.ascontiguousarray(np.asarray(tok_w2, f)[:blocks]
                                 .reshape(blocks, TT, 128, N))
    tokb1 = np.ascontiguousarray(np.asarray(tok_b1, f)[:blocks]
                                 .reshape(blocks, TT, 128).transpose(0, 2, 1))

    g2 = np.asarray(ln2_g, f)[:blocks]
    b2 = np.asarray(ln2_b, f)[:blocks]
    cw1 = np.asarray(ch_w1, f)[:blocks]
    w1g_full = g2[:, :, None] * cw1                                   # (L, C, CH)
    w1g = (w1g_full.reshape(blocks, CT, 128, MT, 128)
           .transpose(0, 3, 2, 1, 4)                 # (L, MT, 128, CT, 128)
           .reshape(blocks, MT // 2, 2, 128, CT, 128)
           .transpose(0, 1, 3, 2, 4, 5))             # (L, 12, 128, 2, CT, 128)
    w1g = np.ascontiguousarray(w1g)
    v = np.einsum("lc,lcm->lm", b2, cw1) + np.asarray(ch_b1, f)[:blocks]
    vb1 = np.ascontiguousarray(v.reshape(blocks, MT, 128).transpose(0, 2, 1))
    chw2 = (np.asarray(ch_w2, f)[:blocks]
            .reshape(blocks, MT // 2, 2, 128, C)
            .transpose(0, 1, 3, 2, 4))               # (L, 12, 128, 2, C)
    chw2 = np.ascontiguousarray(chw2)
    chb2c = np.ascontiguousarray(np.asarray(ch_b2, f)[:blocks]
                                 .reshape(blocks, CT, 128).transpose(0, 2, 1))

    gf = np.asarray(lnf_g, f)
    bf = np.asarray(lnf_b, f)
    hw = np.asarray(head_w, f)
    import ml_dtypes
    headwg = np.ascontiguousarray(
        (gf[:, None] * hw).reshape(CT, 128, K).astype(ml_dtypes.bfloat16))
    headb = (bf @ hw + np.asarray(head_b, f)).reshape(1, K).astype(f)

    ln1g = np.ascontiguousarray(np.asarray(ln1_g, f)[:blocks])
    ln1b = np.ascontiguousarray(np.asarray(ln1_b, f)[:blocks])
    has_g1 = not np.all(ln1g == 1.0)
    has_b1 = not np.all(ln1b == 0.0)

    shared = dict(wq=wq, tokw1c=tokw1c, tokw2=tokw2, tokb1=tokb1, w1g=w1g,
                  vb1=vb1, chw2=chw2, chb2c=chb2c, headwg=headwg, headb=headb,
                  ln1g=ln1g, ln1b=ln1b)
    shared = {k: (np.ascontiguousarray(v) if v.dtype != np.float32 and
                  k == "headwg" else np.ascontiguousarray(v, f))
              for k, v in shared.items()}

    per_core = []
    for c in range(NCORES):
        sel = ptA[c * IPC:(c + 1) * IPC][:items]  # (items, 9, 256)
        ptc = np.ascontiguousarray(sel.transpose(1, 0, 2).reshape(9, items * N))
        per_core.append(dict(pt=ptc))
    return shared, per_core, dict(has_g1=has_g1, has_b1=has_b1)


_CACHE = {}


def kernel(**inputs):
    from concourse.bass_utils import run_bass_kernel_spmd
    shared, per_core, flags = prep_inputs(**inputs)
    key = (flags["has_g1"], flags["has_b1"])
    if key not in _CACHE:
        _CACHE[key] = build(has_g1=flags["has_g1"], has_b1=flags["has_b1"])
    nc = _CACHE[key]
    in_maps = [{**shared, **pc} for pc in per_core]
    res = run_bass_kernel_spmd(nc, in_maps, core_ids=list(range(NCORES)))
    outs = [r["out"] for r in res.results]
    full = np.concatenate(outs, axis=0).astype(np.float32)
    return full + shared["headb"].astype(np.float32)

